# Initial kernel scaffold
#
"""Trainium2 Bass kernel: Gauss-Newton/ADMM x-update for 3-param IR-T1 model.

Self-contained: hardcodes shapes from the problem spec.
  x, z, beta: [16, 256, 256, 3] f32   (A, B, R1 interleaved innermost)
  rho, sigma: [1] f32                 (sigma unused by the reference)
  b:          [16, 256, 256, 8] f32
  tau:        [1, 8] f32
Returns [16, 256, 256, 3] f32.

Strategy: pure data parallel over the 1M pixels -> 8 NeuronCores.
Per core: 131072 pixels laid out as [128 partitions, 1024 free], processed
in chunks of 512 pixels/partition. All per-pixel math is done with planar
[128, n] fp32 planes: 8 exp's on ScalarE, tensor-tensor work on VectorE,
3x3 solve via the adjugate (H is SPD: JTJ + rho*I with rho>0, so the
det<=0 regularization branch of the reference is dead code).

tau and rho are read from the actual input values at build time and baked
into the program as immediates.
"""

import numpy as np

import concourse.bass as bass
import concourse.mybir as mybir
from concourse.tile import TileContext
from concourse.bass_utils import run_bass_kernel_spmd

F32 = mybir.dt.float32
ALU = mybir.AluOpType
ACTF = mybir.ActivationFunctionType

NB, NY, NX, NP, NQ = 16, 256, 256, 3, 8
NCORES = 8
PIX = NB * NY * NX           # 1048576
PIX_CORE = PIX // NCORES     # 131072
PARTS = 128
NFREE = PIX_CORE // PARTS    # 1024
CHUNK = 512                  # pixels per partition per chunk
NCHUNK = NFREE // CHUNK      # 2

# walrus rejects >1 semaphore wait on one instruction; Tile's final drain
# carries one wait per outstanding proc. Split the excess onto NoOps.
_MAX_WAITS = 1

LAST_RESULTS = None  # BassKernelResults of the most recent run (for test.py)


def _split_excess_waits(nc):
    for f in nc.m.functions:
        for blk in f.blocks:
            new_insts = []
            for ins in blk.instructions:
                si = getattr(ins, "sync_info", None)
                if si is not None and si.on_wait and len(si.on_wait) > _MAX_WAITS:
                    waits = list(si.on_wait)
                    extra, keep = waits[:-_MAX_WAITS], waits[-_MAX_WAITS:]
                    for idx, w in enumerate(extra):
                        new_insts.append(
                            mybir.InstNoOp(
                                name=f"{ins.name}-ws{idx}",
                                engine=ins.engine,
                                sync_info=mybir.SyncInfo(on_wait=[w], on_update=[]),
                                bass_nofuse=True,
                            )
                        )
                    si.on_wait = keep
                new_insts.append(ins)
            blk.instructions = new_insts


def _build3(tau, rho, reps=1):
    """v3: minimize blocking semaphore waits (each costs ~45us on this system).

    Rules: ScalarE (ACT) runs only Exp/Square ops that read the input tile or
    ACT's own outputs, into per-chunk-parity plane sets (no WAR stalls within
    a rep). VectorE does everything else on fixed preallocated planes —
    same-engine ordering is free. Inputs are DMA'd once up front; output
    stored once at the end.
    """
    tau = [float(t) for t in tau]
    rho = float(rho)
    c0 = 8.0 + rho  # H[0,0]

    nc = bass.Bass()
    xd = nc.declare_dram_parameter("x", [PIX_CORE, NP], F32, isOutput=False)
    zd = nc.declare_dram_parameter("z", [PIX_CORE, NP], F32, isOutput=False)
    betad = nc.declare_dram_parameter("beta", [PIX_CORE, NP], F32, isOutput=False)
    bd = nc.declare_dram_parameter("b", [PIX_CORE, NQ], F32, isOutput=False)
    yd = nc.declare_dram_parameter("y", [PIX_CORE, NP], F32, isOutput=True)

    xr = xd.rearrange("(p f) c -> p f c", p=PARTS)
    zr = zd.rearrange("(p f) c -> p f c", p=PARTS)
    betar = betad.rearrange("(p f) c -> p f c", p=PARTS)
    br = bd.rearrange("(p f) q -> p f q", p=PARTS)
    yr = yd.rearrange("(p f) c -> p f c", p=PARTS)
    chkd = None
    if reps > 1:
        chkd = nc.declare_dram_parameter("chk", [PARTS, CHUNK], F32, isOutput=True)

    v = nc.vector
    a = nc.scalar

    with TileContext(nc) as tc:
        with tc.tile_pool(name="all", bufs=1) as pool:
            def plane(name):
                return pool.tile([PARTS, CHUNK], F32, tag=name, name=name)

            # resident inputs / output
            xin_g = pool.tile([PARTS, NFREE, NP], F32, tag="xin", name="xin_g")
            zin_g = pool.tile([PARTS, NFREE, NP], F32, tag="zin", name="zin_g")
            betain_g = pool.tile([PARTS, NFREE, NP], F32, tag="betain", name="betain_g")
            bin_g = pool.tile([PARTS, NFREE, NQ], F32, tag="bin", name="bin_g")
            yout_g = pool.tile([PARTS, NFREE, NP], F32, tag="yout", name="yout_g")
            nc.sync.dma_start(xin_g[:], xr[:, :, :])
            nc.sync.dma_start(zin_g[:], zr[:, :, :])
            nc.sync.dma_start(betain_g[:], betar[:, :, :])
            nc.sync.dma_start(bin_g[:], br[:, :, :])

            # ACT-written plane sets, one per chunk parity
            E = [[plane(f"E{s}_{q}") for q in range(NQ)] for s in range(2)]
            E2 = [[plane(f"E2{s}_{q}") for q in range(NQ - 1)] for s in range(2)]

            # accumulators (DVE-private)
            SE, StE, SE2, StE2, St2E2 = (plane(n) for n in ("SE", "StE", "SE2", "StE2", "St2E2"))
            SbE, SbtE, Sb = plane("SbE"), plane("SbtE"), plane("Sb")
            # persistents
            q2, f2, e, h = plane("q2"), plane("f2"), plane("e"), plane("h")
            C00, C01, C02 = plane("C00"), plane("C01"), plane("C02")
            C11, C12, C22 = plane("C11"), plane("C12"), plane("C22")
            invdet = plane("invdet")
            w0, w1, w2 = plane("w0"), plane("w1"), plane("w2")
            g0, g1, g2 = plane("g0"), plane("g1"), plane("g2")
            # scratch (DVE-private, reused in place)
            ta, tb, tc_, td, te, tf = (plane(n) for n in ("ta", "tb", "tc", "td", "te", "tf"))
            Eb = plane("Eb")

            chk = None
            if reps > 1:
                chk = plane("chk")
                v.memset(chk[:], 0.0)

            for it in range(NCHUNK * reps):
                k = it % NCHUNK
                s = k % 2
                sl = slice(k * CHUNK, (k + 1) * CHUNK)
                xin = xin_g[:, sl, :]
                zin = zin_g[:, sl, :]
                betain = betain_g[:, sl, :]
                bin_ = bin_g[:, sl, :]
                yout = yout_g[:, sl, :]
                Av, Bv, R1v = xin[:, :, 0], xin[:, :, 1], xin[:, :, 2]
                Es, E2s = E[s], E2[s]

                # ---- ACT program: exps + squares only ----
                for q in range(NQ):
                    a.activation(Es[q][:], R1v, ACTF.Exp, scale=-tau[q])
                for q in range(1, NQ):
                    a.activation(E2s[q - 1][:], Es[q][:], ACTF.Square)

                # ---- DVE: sums over q ----
                v.tensor_reduce(Sb[:], bin_[:], mybir.AxisListType.X, ALU.add)
                v.tensor_scalar_mul(StE[:], Es[0][:], tau[0])
                # SE2/StE2/St2E2 from E2 of q>=1 plus E0^2 handled via DVE mul
                v.tensor_mul(SE2[:], Es[0][:], Es[0][:])  # E0^2
                v.tensor_scalar_mul(StE2[:], SE2[:], tau[0])
                v.tensor_scalar_mul(St2E2[:], SE2[:], tau[0] * tau[0])
                v.tensor_add(SE[:], Es[0][:], Es[1][:])
                v.tensor_mul(Eb[:], Es[0][:], bin_[:, :, 0])
                v.tensor_copy(SbE[:], Eb[:])
                v.tensor_scalar_mul(SbtE[:], Eb[:], tau[0])
                for q in range(1, NQ):
                    tq = tau[q]
                    E2q = E2s[q - 1]
                    if q > 1:
                        v.tensor_add(SE[:], SE[:], Es[q][:])
                    v.scalar_tensor_tensor(StE[:], Es[q][:], tq, StE[:], ALU.mult, ALU.add)
                    v.tensor_add(SE2[:], SE2[:], E2q[:])
                    v.scalar_tensor_tensor(StE2[:], E2q[:], tq, StE2[:], ALU.mult, ALU.add)
                    v.scalar_tensor_tensor(St2E2[:], E2q[:], tq * tq, St2E2[:], ALU.mult, ALU.add)
                    v.tensor_mul(Eb[:], Es[q][:], bin_[:, :, q])
                    v.tensor_add(SbE[:], SbE[:], Eb[:])
                    v.scalar_tensor_tensor(SbtE[:], Eb[:], tq, SbtE[:], ALU.mult, ALU.add)

                # ---- H entries ----
                v.tensor_mul(q2[:], Bv, StE[:])
                v.tensor_mul(f2[:], Bv, StE2[:])
                v.tensor_scalar_add(e[:], SE2[:], rho)
                v.tensor_mul(ta[:], Bv, Bv)           # B^2
                v.tensor_mul(tb[:], ta[:], St2E2[:])
                v.tensor_scalar_add(h[:], tb[:], rho)

                # ---- cofactors ----
                v.tensor_mul(ta[:], e[:], h[:])
                v.tensor_mul(tb[:], f2[:], f2[:])
                v.tensor_sub(C00[:], ta[:], tb[:])
                v.tensor_mul(ta[:], SE[:], h[:])
                v.tensor_mul(tb[:], f2[:], q2[:])
                v.tensor_sub(C01[:], ta[:], tb[:])
                v.tensor_mul(ta[:], SE[:], f2[:])
                v.tensor_mul(tb[:], e[:], q2[:])
                v.tensor_sub(C02[:], ta[:], tb[:])
                v.tensor_mul(ta[:], q2[:], q2[:])
                v.scalar_tensor_tensor(C11[:], h[:], c0, ta[:], ALU.mult, ALU.subtract)
                v.tensor_mul(ta[:], SE[:], q2[:])
                v.scalar_tensor_tensor(C12[:], f2[:], c0, ta[:], ALU.mult, ALU.subtract)
                v.tensor_mul(ta[:], SE[:], SE[:])
                v.scalar_tensor_tensor(C22[:], e[:], c0, ta[:], ALU.mult, ALU.subtract)

                # ---- det, 1/det ----
                v.tensor_mul(ta[:], SE[:], C01[:])
                v.scalar_tensor_tensor(tb[:], C00[:], c0, ta[:], ALU.mult, ALU.subtract)
                v.tensor_mul(ta[:], q2[:], C02[:])
                v.tensor_add(tb[:], tb[:], ta[:])
                v.reciprocal(invdet[:], tb[:])

                # ---- gradient ----
                v.tensor_sub(w0[:], betain[:, :, 0], zin[:, :, 0])
                v.tensor_sub(w1[:], betain[:, :, 1], zin[:, :, 1])
                v.tensor_sub(w2[:], betain[:, :, 2], zin[:, :, 2])

                v.tensor_add(ta[:], Av, w0[:])        # A + w0
                v.tensor_scalar_mul(ta[:], ta[:], rho)
                v.tensor_mul(tb[:], Bv, SE[:])
                v.tensor_add(tb[:], tb[:], Sb[:])
                v.scalar_tensor_tensor(ta[:], tb[:], -0.125, ta[:], ALU.mult, ALU.add)
                v.tensor_add(g0[:], ta[:], Av)

                v.tensor_add(ta[:], Bv, w1[:])
                v.tensor_scalar_mul(ta[:], ta[:], rho)
                v.tensor_mul(tb[:], Av, SE[:])
                v.tensor_mul(tc_[:], Bv, SE2[:])
                v.tensor_sub(tb[:], tb[:], tc_[:])
                v.tensor_sub(tb[:], tb[:], SbE[:])
                v.scalar_tensor_tensor(g1[:], tb[:], -0.125, ta[:], ALU.mult, ALU.add)

                v.tensor_add(ta[:], R1v, w2[:])
                v.tensor_scalar_mul(ta[:], ta[:], rho)
                v.tensor_mul(tb[:], Av, StE[:])
                v.tensor_sub(tb[:], tb[:], f2[:])
                v.tensor_sub(tb[:], tb[:], SbtE[:])
                v.tensor_mul(tb[:], Bv, tb[:])
                v.scalar_tensor_tensor(g2[:], tb[:], 0.125, ta[:], ALU.mult, ALU.add)

                # ---- d = (C/det) @ g ; out = x - d ----
                v.tensor_mul(g0[:], g0[:], invdet[:])
                v.tensor_mul(g1[:], g1[:], invdet[:])
                v.tensor_mul(g2[:], g2[:], invdet[:])

                for c, (Ca, Cb, Cc) in enumerate(
                    [(C00, C01, C02), (C01, C11, C12), (C02, C12, C22)]
                ):
                    v.tensor_mul(ta[:], Ca[:], g0[:])
                    v.tensor_mul(tb[:], Cb[:], g1[:])
                    v.tensor_add(ta[:], ta[:], tb[:])
                    v.tensor_mul(tb[:], Cc[:], g2[:])
                    v.tensor_add(ta[:], ta[:], tb[:])
                    v.tensor_sub(yout[:, :, c], xin[:, :, c], ta[:])
                    if chk is not None and c == 2:
                        v.tensor_add(chk[:], chk[:], ta[:])

                # store this chunk's output now (runs on DMA engines,
                # overlapping the next chunk's compute). Only on the final
                # rep so timing builds don't multiply store traffic.
                if it // NCHUNK == reps - 1:
                    nc.sync.dma_start(yr[:, sl, :], yout[:])

            if chk is not None:
                nc.sync.dma_start(chkd[:], chk[:])

    _split_excess_waits(nc)
    return nc


def _build(tau, rho, reps=1, phase="full"):
    """Build the per-core Bass program. tau: 8 python floats, rho: float.

    reps>1 repeats the whole computation (identical output) — used only for
    device-time measurement via wall-clock deltas. phase in
    {"full", "sums", "exps", "noact"} truncates the computation for bisection
    benchmarks (output is garbage for phase != "full").
    """
    tau = [float(t) for t in tau]
    rho = float(rho)
    c0 = 8.0 + rho  # H[0,0]

    nc = bass.Bass()
    xd = nc.declare_dram_parameter("x", [PIX_CORE, NP], F32, isOutput=False)
    zd = nc.declare_dram_parameter("z", [PIX_CORE, NP], F32, isOutput=False)
    betad = nc.declare_dram_parameter("beta", [PIX_CORE, NP], F32, isOutput=False)
    bd = nc.declare_dram_parameter("b", [PIX_CORE, NQ], F32, isOutput=False)
    yd = nc.declare_dram_parameter("y", [PIX_CORE, NP], F32, isOutput=True)

    xr = xd.rearrange("(p f) c -> p f c", p=PARTS)
    zr = zd.rearrange("(p f) c -> p f c", p=PARTS)
    betar = betad.rearrange("(p f) c -> p f c", p=PARTS)
    br = bd.rearrange("(p f) q -> p f q", p=PARTS)
    yr = yd.rearrange("(p f) c -> p f c", p=PARTS)
    # reps>1 (timing builds only): a live checksum chain defeats dead-code
    # elimination of the repeated iterations.
    chkd = None
    if reps > 1:
        chkd = nc.declare_dram_parameter("chk", [PARTS, CHUNK], F32, isOutput=True)

    v = nc.vector
    a = nc.scalar

    with TileContext(nc) as tc:
        with (
            tc.tile_pool(name="io", bufs=1) as io,
            tc.tile_pool(name="ering", bufs=3) as ering,
            tc.tile_pool(name="e2ring", bufs=2) as e2ring,
            tc.tile_pool(name="ebring", bufs=2) as ebring,
            tc.tile_pool(name="accs", bufs=2) as accs,
            tc.tile_pool(name="hphase", bufs=1) as hp,
            tc.tile_pool(name="tmp", bufs=10) as tmpp,
        ):
            chk = None
            if reps > 1:
                chk = io.tile([PARTS, CHUNK], F32, tag="chk", name="chk", bufs=1)
                nc.vector.memset(chk[:], 0.0)

            # All inputs resident in SBUF: one DMA per tensor up front,
            # one output store at the end. (Interleaving per-chunk DMAs with
            # the compute stream measured ~2-4 ms per DMA on this system.)
            xin_g = io.tile([PARTS, NFREE, NP], F32, tag="xin", name="xin_g")
            zin_g = io.tile([PARTS, NFREE, NP], F32, tag="zin", name="zin_g")
            betain_g = io.tile([PARTS, NFREE, NP], F32, tag="betain", name="betain_g")
            bin_g = io.tile([PARTS, NFREE, NQ], F32, tag="bin", name="bin_g")
            yout_g = io.tile([PARTS, NFREE, NP], F32, tag="yout", name="yout_g")
            nc.sync.dma_start(xin_g[:], xr[:, :, :])
            nc.sync.dma_start(zin_g[:], zr[:, :, :])
            nc.sync.dma_start(betain_g[:], betar[:, :, :])
            nc.sync.dma_start(bin_g[:], br[:, :, :])

            for k in range(NCHUNK * reps):
                k = k % NCHUNK
                sl = slice(k * CHUNK, (k + 1) * CHUNK)

                xin = xin_g[:, sl, :]
                zin = zin_g[:, sl, :]
                betain = betain_g[:, sl, :]
                bin_ = bin_g[:, sl, :]
                yout = yout_g[:, sl, :]

                Av = xin[:, :, 0]
                Bv = xin[:, :, 1]
                R1v = xin[:, :, 2]

                def plane(pool, tag):
                    return pool.tile([PARTS, CHUNK], F32, tag=tag, name=tag)

                yflat = yout.rearrange("p f c -> p (f c)")

                if phase == "exps":
                    chkE = plane(accs, "SE")
                    for q in range(NQ):
                        E = plane(ering, "E")
                        a.activation(E[:], R1v, ACTF.Exp, scale=-tau[q])
                        if q == 0:
                            a.copy(chkE[:], E[:])
                        else:
                            v.tensor_add(chkE[:], chkE[:], E[:])
                    v.tensor_copy(yflat[:, 0:CHUNK], chkE[:])
                    if chk is not None:
                        v.tensor_add(chk[:], chk[:], chkE[:])
                    continue

                # ---- sums over q ----
                SE = plane(accs, "SE")
                StE = plane(accs, "StE")
                SE2 = plane(accs, "SE2")
                StE2 = plane(accs, "StE2")
                St2E2 = plane(accs, "St2E2")
                SbE = plane(accs, "SbE")
                SbtE = plane(accs, "SbtE")
                Sb = plane(accs, "Sb")

                v.tensor_reduce(Sb[:], bin_[:], mybir.AxisListType.X, ALU.add)

                for q in range(NQ):
                    tq = tau[q]
                    E = plane(ering, "E")
                    a.activation(E[:], R1v, ACTF.Exp, scale=-tq)
                    Eb = plane(ebring, "Eb")
                    v.tensor_mul(Eb[:], E[:], bin_[:, :, q])
                    if q == 0:
                        a.copy(SE[:], E[:])
                        a.mul(StE[:], E[:], tq)
                        a.activation(SE2[:], E[:], ACTF.Square)
                        a.activation(StE2[:], E[:], ACTF.Square, scale=float(np.sqrt(tq)))
                        a.activation(St2E2[:], E[:], ACTF.Square, scale=tq)
                        a.copy(SbE[:], Eb[:])
                        a.mul(SbtE[:], Eb[:], tq)
                    else:
                        E2 = plane(e2ring, "E2")
                        a.activation(E2[:], E[:], ACTF.Square)
                        v.tensor_add(SE[:], SE[:], E[:])
                        v.scalar_tensor_tensor(StE[:], E[:], tq, StE[:], ALU.mult, ALU.add)
                        v.tensor_add(SE2[:], SE2[:], E2[:])
                        v.scalar_tensor_tensor(StE2[:], E2[:], tq, StE2[:], ALU.mult, ALU.add)
                        v.scalar_tensor_tensor(St2E2[:], E2[:], tq * tq, St2E2[:], ALU.mult, ALU.add)
                        v.tensor_add(SbE[:], SbE[:], Eb[:])
                        v.scalar_tensor_tensor(SbtE[:], Eb[:], tq, SbtE[:], ALU.mult, ALU.add)

                if phase == "sums":
                    sm = plane(tmpp, "tmp")
                    v.tensor_add(sm[:], SE[:], StE[:])
                    for other in (SE2, StE2, St2E2, SbE, SbtE, Sb):
                        v.tensor_add(sm[:], sm[:], other[:])
                    v.tensor_copy(yflat[:, 0:CHUNK], sm[:])
                    if chk is not None:
                        v.tensor_add(chk[:], chk[:], sm[:])
                    continue

                # ---- H entries (c0=8+rho baked) ----
                # H = [[c0, -p, q2], [-p, e, -f2], [q2, -f2, h]]
                #  p=SE, q2=B*StE, e=SE2+rho, f2=B*StE2, h=B^2*St2E2+rho
                q2 = plane(hp, "q2")
                f2 = plane(hp, "f2")
                e = plane(hp, "e")
                h = plane(hp, "h")
                v.tensor_mul(q2[:], Bv, StE[:])
                v.tensor_mul(f2[:], Bv, StE2[:])
                v.tensor_scalar_add(e[:], SE2[:], rho)
                B2 = plane(tmpp, "tmp")
                a.activation(B2[:], Bv, ACTF.Square)
                t0 = plane(tmpp, "tmp")
                v.tensor_mul(t0[:], B2[:], St2E2[:])
                v.tensor_scalar_add(h[:], t0[:], rho)

                # ---- cofactors ----
                f2sq = plane(tmpp, "tmp")
                q2sq = plane(tmpp, "tmp")
                psq = plane(tmpp, "tmp")
                a.activation(f2sq[:], f2[:], ACTF.Square)
                a.activation(q2sq[:], q2[:], ACTF.Square)
                a.activation(psq[:], SE[:], ACTF.Square)

                C00 = plane(hp, "C00")
                C01 = plane(hp, "C01")
                C02 = plane(hp, "C02")
                C11 = plane(hp, "C11")
                C12 = plane(hp, "C12")
                C22 = plane(hp, "C22")

                t1 = plane(tmpp, "tmp")
                v.tensor_mul(t1[:], e[:], h[:])
                v.tensor_sub(C00[:], t1[:], f2sq[:])

                t2 = plane(tmpp, "tmp")
                t3 = plane(tmpp, "tmp")
                v.tensor_mul(t2[:], SE[:], h[:])
                v.tensor_mul(t3[:], f2[:], q2[:])
                v.tensor_sub(C01[:], t2[:], t3[:])

                t4 = plane(tmpp, "tmp")
                t5 = plane(tmpp, "tmp")
                v.tensor_mul(t4[:], SE[:], f2[:])
                v.tensor_mul(t5[:], e[:], q2[:])
                v.tensor_sub(C02[:], t4[:], t5[:])

                v.scalar_tensor_tensor(C11[:], h[:], c0, q2sq[:], ALU.mult, ALU.subtract)
                t6 = plane(tmpp, "tmp")
                v.tensor_mul(t6[:], SE[:], q2[:])
                v.scalar_tensor_tensor(C12[:], f2[:], c0, t6[:], ALU.mult, ALU.subtract)
                v.scalar_tensor_tensor(C22[:], e[:], c0, psq[:], ALU.mult, ALU.subtract)

                # ---- det and 1/det ----
                dt1 = plane(tmpp, "tmp")
                dt2 = plane(tmpp, "tmp")
                dt3 = plane(tmpp, "tmp")
                det = plane(hp, "det")
                invdet = plane(hp, "invdet")
                v.tensor_mul(dt1[:], SE[:], C01[:])
                v.scalar_tensor_tensor(dt2[:], C00[:], c0, dt1[:], ALU.mult, ALU.subtract)
                v.tensor_mul(dt3[:], q2[:], C02[:])
                v.tensor_add(det[:], dt2[:], dt3[:])
                v.reciprocal(invdet[:], det[:])

                # ---- gradient ----
                # w_c = beta_c - z_c
                w0 = plane(hp, "w0")
                w1 = plane(hp, "w1")
                w2 = plane(hp, "w2")
                v.tensor_sub(w0[:], betain[:, :, 0], zin[:, :, 0])
                v.tensor_sub(w1[:], betain[:, :, 1], zin[:, :, 1])
                v.tensor_sub(w2[:], betain[:, :, 2], zin[:, :, 2])

                # g0 = A - (B*SE + Sb)/8 + rho*(A + w0)
                g0 = plane(hp, "g0")
                u = plane(tmpp, "tmp")
                ru = plane(tmpp, "tmp")
                v.tensor_add(u[:], Av, w0[:])
                a.mul(ru[:], u[:], rho)
                ta = plane(tmpp, "tmp")
                tb = plane(tmpp, "tmp")
                v.tensor_mul(ta[:], Bv, SE[:])
                v.tensor_add(tb[:], ta[:], Sb[:])
                g0a = plane(tmpp, "tmp")
                v.scalar_tensor_tensor(g0a[:], tb[:], -0.125, ru[:], ALU.mult, ALU.add)
                v.tensor_add(g0[:], g0a[:], Av)

                # g1 = -(A*SE - B*SE2 - SbE)/8 + rho*(B + w1)
                g1 = plane(hp, "g1")
                vb = plane(tmpp, "tmp")
                rv = plane(tmpp, "tmp")
                v.tensor_add(vb[:], Bv, w1[:])
                a.mul(rv[:], vb[:], rho)
                tc1 = plane(tmpp, "tmp")
                tc2 = plane(tmpp, "tmp")
                v.tensor_mul(tc1[:], Av, SE[:])
                v.tensor_mul(tc2[:], Bv, SE2[:])
                tc3 = plane(tmpp, "tmp")
                v.tensor_sub(tc3[:], tc1[:], tc2[:])
                tc4 = plane(tmpp, "tmp")
                v.tensor_sub(tc4[:], tc3[:], SbE[:])
                v.scalar_tensor_tensor(g1[:], tc4[:], -0.125, rv[:], ALU.mult, ALU.add)

                # g2 = B*(A*StE - B*StE2 - SbtE)/8 + rho*(R1 + w2)
                g2 = plane(hp, "g2")
                v2t = plane(tmpp, "tmp")
                rv2 = plane(tmpp, "tmp")
                v.tensor_add(v2t[:], R1v, w2[:])
                a.mul(rv2[:], v2t[:], rho)
                td1 = plane(tmpp, "tmp")
                v.tensor_mul(td1[:], Av, StE[:])
                td2 = plane(tmpp, "tmp")
                v.tensor_sub(td2[:], td1[:], f2[:])
                td3 = plane(tmpp, "tmp")
                v.tensor_sub(td3[:], td2[:], SbtE[:])
                td4 = plane(tmpp, "tmp")
                v.tensor_mul(td4[:], Bv, td3[:])
                v.scalar_tensor_tensor(g2[:], td4[:], 0.125, rv2[:], ALU.mult, ALU.add)

                # ---- d = (C/det) @ g ;  out = x - d ----
                g0s = plane(hp, "g0s")
                g1s = plane(hp, "g1s")
                g2s = plane(hp, "g2s")
                v.tensor_mul(g0s[:], g0[:], invdet[:])
                v.tensor_mul(g1s[:], g1[:], invdet[:])
                v.tensor_mul(g2s[:], g2[:], invdet[:])

                for c, (Ca, Cb, Cc) in enumerate(
                    [(C00, C01, C02), (C01, C11, C12), (C02, C12, C22)]
                ):
                    m0 = plane(tmpp, "tmp")
                    m1 = plane(tmpp, "tmp")
                    m2 = plane(tmpp, "tmp")
                    v.tensor_mul(m0[:], Ca[:], g0s[:])
                    v.tensor_mul(m1[:], Cb[:], g1s[:])
                    v.tensor_mul(m2[:], Cc[:], g2s[:])
                    s0 = plane(tmpp, "tmp")
                    v.tensor_add(s0[:], m0[:], m1[:])
                    dsum = plane(tmpp, "tmp")
                    v.tensor_add(dsum[:], s0[:], m2[:])
                    v.tensor_sub(yout[:, :, c], xin[:, :, c], dsum[:])
                    if chk is not None and c == 2:
                        v.tensor_add(chk[:], chk[:], dsum[:])

            nc.sync.dma_start(yr[:, :, :], yout_g[:])
            if chk is not None:
                nc.sync.dma_start(chkd[:], chk[:])

    _split_excess_waits(nc)
    return nc


def kernel(x, z, beta, rho, sigma, b, tau):
    global LAST_RESULTS
    x = np.ascontiguousarray(np.asarray(x, dtype=np.float32).reshape(PIX, NP))
    z = np.ascontiguousarray(np.asarray(z, dtype=np.float32).reshape(PIX, NP))
    beta = np.ascontiguousarray(np.asarray(beta, dtype=np.float32).reshape(PIX, NP))
    b = np.ascontiguousarray(np.asarray(b, dtype=np.float32).reshape(PIX, NQ))
    tau_vals = np.asarray(tau, dtype=np.float32).reshape(NQ)
    rho_val = float(np.asarray(rho, dtype=np.float32).reshape(()))

    nc = _build3(tau_vals, rho_val)

    in_maps = []
    for c in range(NCORES):
        sl = slice(c * PIX_CORE, (c + 1) * PIX_CORE)
        in_maps.append(
            {
                "x": np.ascontiguousarray(x[sl]),
                "z": np.ascontiguousarray(z[sl]),
                "beta": np.ascontiguousarray(beta[sl]),
                "b": np.ascontiguousarray(b[sl]),
            }
        )

    res = run_bass_kernel_spmd(nc, in_maps, list(range(NCORES)))
    LAST_RESULTS = res
    y = np.concatenate([res.results[c]["y"] for c in range(NCORES)], axis=0)
    return y.reshape(NB, NY, NX, NP)



# revision 21
# speedup vs baseline: 42.4987x; 42.4987x over previous
"""Trainium2 Bass kernel: Gauss-Newton/ADMM x-update for 3-param IR-T1 model.

Self-contained: hardcodes shapes from the problem spec.
  x, z, beta: [16, 256, 256, 3] f32   (A, B, R1 interleaved innermost)
  rho, sigma: [1] f32                 (sigma unused by the reference)
  b:          [16, 256, 256, 8] f32
  tau:        [1, 8] f32
Returns [16, 256, 256, 3] f32.

Strategy: pure data parallel over the 1M pixels -> 8 NeuronCores, 131072
pixels/core as [128 partitions, 1024 free], one full-width pass (kernel()
uses _build5, the measured-fastest variant at ~158 us/iteration vs ~215
for the previous adjugate/2-chunk baseline).

Per-pixel math, all f32 (bf16 anywhere upstream of the solve is
numerically catastrophic for ill-conditioned H):
  ScalarE (ACT): E_q=exp(-tau_q R1), E2_q=exp(-2 tau_q R1) into 3-deep
    rings, B^2, SE^2, q2^2 squares, +rho biases, chain inits, z0 scaling.
  VectorE (DVE): Eb muls, 8 interleaved accumulator chains (weighted ones
    via scalar_tensor_tensor), H assembly, LDL^T factor/solve (stable for
    SPD H = JTJ + rho I; d1,d2 clamped at rho/2; ~100x more accurate vs
    fp64 truth than the reference's own f32 LU on catastrophic pixels),
    ghat = 8*grad assembly (the 1/8 mean folds into the final
    y = x - d/8 STTs, which overwrite x in SBUF).
  Pool/GPSIMD is deliberately idle: measured ~2.2-2.5 ns/elem vs DVE
    ~1.25-1.5, and any Pool participation in the per-q ring pipeline
    paces ACT/DVE down via ring WAR edges (measured regressions).

tau and rho are read from the actual input values at build time and baked
into the program as immediates. Timing lore (axon tunnel): per-call
dispatch floor ~9-14 ms and NEFF-size-dependent constants corrupt naive
deltas; measure with large-reps programs whose device time exceeds the
floor, via pipelined burst marginals (see timing2.py).
"""

import numpy as np

import concourse.bass as bass
import concourse.mybir as mybir
from concourse.tile import TileContext
from concourse.bass_utils import run_bass_kernel_spmd

F32 = mybir.dt.float32
ALU = mybir.AluOpType
ACTF = mybir.ActivationFunctionType

NB, NY, NX, NP, NQ = 16, 256, 256, 3, 8
NCORES = 8
PIX = NB * NY * NX           # 1048576
PIX_CORE = PIX // NCORES     # 131072
PARTS = 128
NFREE = PIX_CORE // PARTS    # 1024
CHUNK = 512                  # pixels per partition per chunk
NCHUNK = NFREE // CHUNK      # 2

# walrus rejects >1 semaphore wait on one instruction; Tile's final drain
# carries one wait per outstanding proc. Split the excess onto NoOps.
_MAX_WAITS = 1

LAST_RESULTS = None  # BassKernelResults of the most recent run (for test.py)


def _split_excess_waits(nc):
    for f in nc.m.functions:
        for blk in f.blocks:
            new_insts = []
            for ins in blk.instructions:
                si = getattr(ins, "sync_info", None)
                if si is not None and si.on_wait and len(si.on_wait) > _MAX_WAITS:
                    waits = list(si.on_wait)
                    extra, keep = waits[:-_MAX_WAITS], waits[-_MAX_WAITS:]
                    for idx, w in enumerate(extra):
                        new_insts.append(
                            mybir.InstNoOp(
                                name=f"{ins.name}-ws{idx}",
                                engine=ins.engine,
                                sync_info=mybir.SyncInfo(on_wait=[w], on_update=[]),
                                bass_nofuse=True,
                            )
                        )
                    si.on_wait = keep
                new_insts.append(ins)
            blk.instructions = new_insts


def _build3(tau, rho, reps=1):
    """v3: minimize blocking semaphore waits (each costs ~45us on this system).

    Rules: ScalarE (ACT) runs only Exp/Square ops that read the input tile or
    ACT's own outputs, into per-chunk-parity plane sets (no WAR stalls within
    a rep). VectorE does everything else on fixed preallocated planes —
    same-engine ordering is free. Inputs are DMA'd once up front; output
    stored once at the end.
    """
    tau = [float(t) for t in tau]
    rho = float(rho)
    c0 = 8.0 + rho  # H[0,0]

    nc = bass.Bass()
    xd = nc.declare_dram_parameter("x", [PIX_CORE, NP], F32, isOutput=False)
    zd = nc.declare_dram_parameter("z", [PIX_CORE, NP], F32, isOutput=False)
    betad = nc.declare_dram_parameter("beta", [PIX_CORE, NP], F32, isOutput=False)
    bd = nc.declare_dram_parameter("b", [PIX_CORE, NQ], F32, isOutput=False)
    yd = nc.declare_dram_parameter("y", [PIX_CORE, NP], F32, isOutput=True)

    xr = xd.rearrange("(p f) c -> p f c", p=PARTS)
    zr = zd.rearrange("(p f) c -> p f c", p=PARTS)
    betar = betad.rearrange("(p f) c -> p f c", p=PARTS)
    br = bd.rearrange("(p f) q -> p f q", p=PARTS)
    yr = yd.rearrange("(p f) c -> p f c", p=PARTS)
    chkd = None
    if reps > 1:
        chkd = nc.declare_dram_parameter("chk", [PARTS, CHUNK], F32, isOutput=True)

    v = nc.vector
    a = nc.scalar

    with TileContext(nc) as tc:
        with tc.tile_pool(name="all", bufs=1) as pool:
            def plane(name):
                return pool.tile([PARTS, CHUNK], F32, tag=name, name=name)

            # resident inputs / output
            xin_g = pool.tile([PARTS, NFREE, NP], F32, tag="xin", name="xin_g")
            zin_g = pool.tile([PARTS, NFREE, NP], F32, tag="zin", name="zin_g")
            betain_g = pool.tile([PARTS, NFREE, NP], F32, tag="betain", name="betain_g")
            bin_g = pool.tile([PARTS, NFREE, NQ], F32, tag="bin", name="bin_g")
            yout_g = pool.tile([PARTS, NFREE, NP], F32, tag="yout", name="yout_g")
            nc.sync.dma_start(xin_g[:], xr[:, :, :])
            nc.sync.dma_start(zin_g[:], zr[:, :, :])
            nc.sync.dma_start(betain_g[:], betar[:, :, :])
            nc.sync.dma_start(bin_g[:], br[:, :, :])

            # ACT-written plane sets, one per chunk parity
            E = [[plane(f"E{s}_{q}") for q in range(NQ)] for s in range(2)]
            E2 = [[plane(f"E2{s}_{q}") for q in range(NQ - 1)] for s in range(2)]

            # accumulators (DVE-private)
            SE, StE, SE2, StE2, St2E2 = (plane(n) for n in ("SE", "StE", "SE2", "StE2", "St2E2"))
            SbE, SbtE, Sb = plane("SbE"), plane("SbtE"), plane("Sb")
            # persistents
            q2, f2, e, h = plane("q2"), plane("f2"), plane("e"), plane("h")
            C00, C01, C02 = plane("C00"), plane("C01"), plane("C02")
            C11, C12, C22 = plane("C11"), plane("C12"), plane("C22")
            invdet = plane("invdet")
            w0, w1, w2 = plane("w0"), plane("w1"), plane("w2")
            g0, g1, g2 = plane("g0"), plane("g1"), plane("g2")
            # scratch (DVE-private, reused in place)
            ta, tb, tc_, td, te, tf = (plane(n) for n in ("ta", "tb", "tc", "td", "te", "tf"))
            Eb = plane("Eb")

            chk = None
            if reps > 1:
                chk = plane("chk")
                v.memset(chk[:], 0.0)

            for it in range(NCHUNK * reps):
                k = it % NCHUNK
                s = k % 2
                sl = slice(k * CHUNK, (k + 1) * CHUNK)
                xin = xin_g[:, sl, :]
                zin = zin_g[:, sl, :]
                betain = betain_g[:, sl, :]
                bin_ = bin_g[:, sl, :]
                yout = yout_g[:, sl, :]
                Av, Bv, R1v = xin[:, :, 0], xin[:, :, 1], xin[:, :, 2]
                Es, E2s = E[s], E2[s]

                # ---- ACT program: exps + squares only ----
                for q in range(NQ):
                    a.activation(Es[q][:], R1v, ACTF.Exp, scale=-tau[q])
                for q in range(1, NQ):
                    a.activation(E2s[q - 1][:], Es[q][:], ACTF.Square)

                # ---- DVE: sums over q ----
                v.tensor_reduce(Sb[:], bin_[:], mybir.AxisListType.X, ALU.add)
                v.tensor_scalar_mul(StE[:], Es[0][:], tau[0])
                # SE2/StE2/St2E2 from E2 of q>=1 plus E0^2 handled via DVE mul
                v.tensor_mul(SE2[:], Es[0][:], Es[0][:])  # E0^2
                v.tensor_scalar_mul(StE2[:], SE2[:], tau[0])
                v.tensor_scalar_mul(St2E2[:], SE2[:], tau[0] * tau[0])
                v.tensor_add(SE[:], Es[0][:], Es[1][:])
                v.tensor_mul(Eb[:], Es[0][:], bin_[:, :, 0])
                v.tensor_copy(SbE[:], Eb[:])
                v.tensor_scalar_mul(SbtE[:], Eb[:], tau[0])
                for q in range(1, NQ):
                    tq = tau[q]
                    E2q = E2s[q - 1]
                    if q > 1:
                        v.tensor_add(SE[:], SE[:], Es[q][:])
                    v.scalar_tensor_tensor(StE[:], Es[q][:], tq, StE[:], ALU.mult, ALU.add)
                    v.tensor_add(SE2[:], SE2[:], E2q[:])
                    v.scalar_tensor_tensor(StE2[:], E2q[:], tq, StE2[:], ALU.mult, ALU.add)
                    v.scalar_tensor_tensor(St2E2[:], E2q[:], tq * tq, St2E2[:], ALU.mult, ALU.add)
                    v.tensor_mul(Eb[:], Es[q][:], bin_[:, :, q])
                    v.tensor_add(SbE[:], SbE[:], Eb[:])
                    v.scalar_tensor_tensor(SbtE[:], Eb[:], tq, SbtE[:], ALU.mult, ALU.add)

                # ---- H entries ----
                v.tensor_mul(q2[:], Bv, StE[:])
                v.tensor_mul(f2[:], Bv, StE2[:])
                v.tensor_scalar_add(e[:], SE2[:], rho)
                v.tensor_mul(ta[:], Bv, Bv)           # B^2
                v.tensor_mul(tb[:], ta[:], St2E2[:])
                v.tensor_scalar_add(h[:], tb[:], rho)

                # ---- cofactors ----
                v.tensor_mul(ta[:], e[:], h[:])
                v.tensor_mul(tb[:], f2[:], f2[:])
                v.tensor_sub(C00[:], ta[:], tb[:])
                v.tensor_mul(ta[:], SE[:], h[:])
                v.tensor_mul(tb[:], f2[:], q2[:])
                v.tensor_sub(C01[:], ta[:], tb[:])
                v.tensor_mul(ta[:], SE[:], f2[:])
                v.tensor_mul(tb[:], e[:], q2[:])
                v.tensor_sub(C02[:], ta[:], tb[:])
                v.tensor_mul(ta[:], q2[:], q2[:])
                v.scalar_tensor_tensor(C11[:], h[:], c0, ta[:], ALU.mult, ALU.subtract)
                v.tensor_mul(ta[:], SE[:], q2[:])
                v.scalar_tensor_tensor(C12[:], f2[:], c0, ta[:], ALU.mult, ALU.subtract)
                v.tensor_mul(ta[:], SE[:], SE[:])
                v.scalar_tensor_tensor(C22[:], e[:], c0, ta[:], ALU.mult, ALU.subtract)

                # ---- det, 1/det ----
                v.tensor_mul(ta[:], SE[:], C01[:])
                v.scalar_tensor_tensor(tb[:], C00[:], c0, ta[:], ALU.mult, ALU.subtract)
                v.tensor_mul(ta[:], q2[:], C02[:])
                v.tensor_add(tb[:], tb[:], ta[:])
                v.reciprocal(invdet[:], tb[:])

                # ---- gradient ----
                v.tensor_sub(w0[:], betain[:, :, 0], zin[:, :, 0])
                v.tensor_sub(w1[:], betain[:, :, 1], zin[:, :, 1])
                v.tensor_sub(w2[:], betain[:, :, 2], zin[:, :, 2])

                v.tensor_add(ta[:], Av, w0[:])        # A + w0
                v.tensor_scalar_mul(ta[:], ta[:], rho)
                v.tensor_mul(tb[:], Bv, SE[:])
                v.tensor_add(tb[:], tb[:], Sb[:])
                v.scalar_tensor_tensor(ta[:], tb[:], -0.125, ta[:], ALU.mult, ALU.add)
                v.tensor_add(g0[:], ta[:], Av)

                v.tensor_add(ta[:], Bv, w1[:])
                v.tensor_scalar_mul(ta[:], ta[:], rho)
                v.tensor_mul(tb[:], Av, SE[:])
                v.tensor_mul(tc_[:], Bv, SE2[:])
                v.tensor_sub(tb[:], tb[:], tc_[:])
                v.tensor_sub(tb[:], tb[:], SbE[:])
                v.scalar_tensor_tensor(g1[:], tb[:], -0.125, ta[:], ALU.mult, ALU.add)

                v.tensor_add(ta[:], R1v, w2[:])
                v.tensor_scalar_mul(ta[:], ta[:], rho)
                v.tensor_mul(tb[:], Av, StE[:])
                v.tensor_sub(tb[:], tb[:], f2[:])
                v.tensor_sub(tb[:], tb[:], SbtE[:])
                v.tensor_mul(tb[:], Bv, tb[:])
                v.scalar_tensor_tensor(g2[:], tb[:], 0.125, ta[:], ALU.mult, ALU.add)

                # ---- d = (C/det) @ g ; out = x - d ----
                v.tensor_mul(g0[:], g0[:], invdet[:])
                v.tensor_mul(g1[:], g1[:], invdet[:])
                v.tensor_mul(g2[:], g2[:], invdet[:])

                for c, (Ca, Cb, Cc) in enumerate(
                    [(C00, C01, C02), (C01, C11, C12), (C02, C12, C22)]
                ):
                    v.tensor_mul(ta[:], Ca[:], g0[:])
                    v.tensor_mul(tb[:], Cb[:], g1[:])
                    v.tensor_add(ta[:], ta[:], tb[:])
                    v.tensor_mul(tb[:], Cc[:], g2[:])
                    v.tensor_add(ta[:], ta[:], tb[:])
                    v.tensor_sub(yout[:, :, c], xin[:, :, c], ta[:])
                    if chk is not None and c == 2:
                        v.tensor_add(chk[:], chk[:], ta[:])

                # store this chunk's output now (runs on DMA engines,
                # overlapping the next chunk's compute). Only on the final
                # rep so timing builds don't multiply store traffic.
                if it // NCHUNK == reps - 1:
                    nc.sync.dma_start(yr[:, sl, :], yout[:])

            if chk is not None:
                nc.sync.dma_start(chkd[:], chk[:])

    _split_excess_waits(nc)
    return nc


def _build(tau, rho, reps=1, phase="full"):
    """Build the per-core Bass program. tau: 8 python floats, rho: float.

    reps>1 repeats the whole computation (identical output) — used only for
    device-time measurement via wall-clock deltas. phase in
    {"full", "sums", "exps", "noact"} truncates the computation for bisection
    benchmarks (output is garbage for phase != "full").
    """
    tau = [float(t) for t in tau]
    rho = float(rho)
    c0 = 8.0 + rho  # H[0,0]

    nc = bass.Bass()
    xd = nc.declare_dram_parameter("x", [PIX_CORE, NP], F32, isOutput=False)
    zd = nc.declare_dram_parameter("z", [PIX_CORE, NP], F32, isOutput=False)
    betad = nc.declare_dram_parameter("beta", [PIX_CORE, NP], F32, isOutput=False)
    bd = nc.declare_dram_parameter("b", [PIX_CORE, NQ], F32, isOutput=False)
    yd = nc.declare_dram_parameter("y", [PIX_CORE, NP], F32, isOutput=True)

    xr = xd.rearrange("(p f) c -> p f c", p=PARTS)
    zr = zd.rearrange("(p f) c -> p f c", p=PARTS)
    betar = betad.rearrange("(p f) c -> p f c", p=PARTS)
    br = bd.rearrange("(p f) q -> p f q", p=PARTS)
    yr = yd.rearrange("(p f) c -> p f c", p=PARTS)
    # reps>1 (timing builds only): a live checksum chain defeats dead-code
    # elimination of the repeated iterations.
    chkd = None
    if reps > 1:
        chkd = nc.declare_dram_parameter("chk", [PARTS, CHUNK], F32, isOutput=True)

    v = nc.vector
    a = nc.scalar

    with TileContext(nc) as tc:
        with (
            tc.tile_pool(name="io", bufs=1) as io,
            tc.tile_pool(name="ering", bufs=3) as ering,
            tc.tile_pool(name="e2ring", bufs=2) as e2ring,
            tc.tile_pool(name="ebring", bufs=2) as ebring,
            tc.tile_pool(name="accs", bufs=2) as accs,
            tc.tile_pool(name="hphase", bufs=1) as hp,
            tc.tile_pool(name="tmp", bufs=10) as tmpp,
        ):
            chk = None
            if reps > 1:
                chk = io.tile([PARTS, CHUNK], F32, tag="chk", name="chk", bufs=1)
                nc.vector.memset(chk[:], 0.0)

            # All inputs resident in SBUF: one DMA per tensor up front,
            # one output store at the end. (Interleaving per-chunk DMAs with
            # the compute stream measured ~2-4 ms per DMA on this system.)
            xin_g = io.tile([PARTS, NFREE, NP], F32, tag="xin", name="xin_g")
            zin_g = io.tile([PARTS, NFREE, NP], F32, tag="zin", name="zin_g")
            betain_g = io.tile([PARTS, NFREE, NP], F32, tag="betain", name="betain_g")
            bin_g = io.tile([PARTS, NFREE, NQ], F32, tag="bin", name="bin_g")
            yout_g = io.tile([PARTS, NFREE, NP], F32, tag="yout", name="yout_g")
            nc.sync.dma_start(xin_g[:], xr[:, :, :])
            nc.sync.dma_start(zin_g[:], zr[:, :, :])
            nc.sync.dma_start(betain_g[:], betar[:, :, :])
            nc.sync.dma_start(bin_g[:], br[:, :, :])

            for k in range(NCHUNK * reps):
                k = k % NCHUNK
                sl = slice(k * CHUNK, (k + 1) * CHUNK)

                xin = xin_g[:, sl, :]
                zin = zin_g[:, sl, :]
                betain = betain_g[:, sl, :]
                bin_ = bin_g[:, sl, :]
                yout = yout_g[:, sl, :]

                Av = xin[:, :, 0]
                Bv = xin[:, :, 1]
                R1v = xin[:, :, 2]

                def plane(pool, tag):
                    return pool.tile([PARTS, CHUNK], F32, tag=tag, name=tag)

                yflat = yout.rearrange("p f c -> p (f c)")

                if phase == "exps":
                    chkE = plane(accs, "SE")
                    for q in range(NQ):
                        E = plane(ering, "E")
                        a.activation(E[:], R1v, ACTF.Exp, scale=-tau[q])
                        if q == 0:
                            a.copy(chkE[:], E[:])
                        else:
                            v.tensor_add(chkE[:], chkE[:], E[:])
                    v.tensor_copy(yflat[:, 0:CHUNK], chkE[:])
                    if chk is not None:
                        v.tensor_add(chk[:], chk[:], chkE[:])
                    continue

                # ---- sums over q ----
                SE = plane(accs, "SE")
                StE = plane(accs, "StE")
                SE2 = plane(accs, "SE2")
                StE2 = plane(accs, "StE2")
                St2E2 = plane(accs, "St2E2")
                SbE = plane(accs, "SbE")
                SbtE = plane(accs, "SbtE")
                Sb = plane(accs, "Sb")

                v.tensor_reduce(Sb[:], bin_[:], mybir.AxisListType.X, ALU.add)

                for q in range(NQ):
                    tq = tau[q]
                    E = plane(ering, "E")
                    a.activation(E[:], R1v, ACTF.Exp, scale=-tq)
                    Eb = plane(ebring, "Eb")
                    v.tensor_mul(Eb[:], E[:], bin_[:, :, q])
                    if q == 0:
                        a.copy(SE[:], E[:])
                        a.mul(StE[:], E[:], tq)
                        a.activation(SE2[:], E[:], ACTF.Square)
                        a.activation(StE2[:], E[:], ACTF.Square, scale=float(np.sqrt(tq)))
                        a.activation(St2E2[:], E[:], ACTF.Square, scale=tq)
                        a.copy(SbE[:], Eb[:])
                        a.mul(SbtE[:], Eb[:], tq)
                    else:
                        E2 = plane(e2ring, "E2")
                        a.activation(E2[:], E[:], ACTF.Square)
                        v.tensor_add(SE[:], SE[:], E[:])
                        v.scalar_tensor_tensor(StE[:], E[:], tq, StE[:], ALU.mult, ALU.add)
                        v.tensor_add(SE2[:], SE2[:], E2[:])
                        v.scalar_tensor_tensor(StE2[:], E2[:], tq, StE2[:], ALU.mult, ALU.add)
                        v.scalar_tensor_tensor(St2E2[:], E2[:], tq * tq, St2E2[:], ALU.mult, ALU.add)
                        v.tensor_add(SbE[:], SbE[:], Eb[:])
                        v.scalar_tensor_tensor(SbtE[:], Eb[:], tq, SbtE[:], ALU.mult, ALU.add)

                if phase == "sums":
                    sm = plane(tmpp, "tmp")
                    v.tensor_add(sm[:], SE[:], StE[:])
                    for other in (SE2, StE2, St2E2, SbE, SbtE, Sb):
                        v.tensor_add(sm[:], sm[:], other[:])
                    v.tensor_copy(yflat[:, 0:CHUNK], sm[:])
                    if chk is not None:
                        v.tensor_add(chk[:], chk[:], sm[:])
                    continue

                # ---- H entries (c0=8+rho baked) ----
                # H = [[c0, -p, q2], [-p, e, -f2], [q2, -f2, h]]
                #  p=SE, q2=B*StE, e=SE2+rho, f2=B*StE2, h=B^2*St2E2+rho
                q2 = plane(hp, "q2")
                f2 = plane(hp, "f2")
                e = plane(hp, "e")
                h = plane(hp, "h")
                v.tensor_mul(q2[:], Bv, StE[:])
                v.tensor_mul(f2[:], Bv, StE2[:])
                v.tensor_scalar_add(e[:], SE2[:], rho)
                B2 = plane(tmpp, "tmp")
                a.activation(B2[:], Bv, ACTF.Square)
                t0 = plane(tmpp, "tmp")
                v.tensor_mul(t0[:], B2[:], St2E2[:])
                v.tensor_scalar_add(h[:], t0[:], rho)

                # ---- cofactors ----
                f2sq = plane(tmpp, "tmp")
                q2sq = plane(tmpp, "tmp")
                psq = plane(tmpp, "tmp")
                a.activation(f2sq[:], f2[:], ACTF.Square)
                a.activation(q2sq[:], q2[:], ACTF.Square)
                a.activation(psq[:], SE[:], ACTF.Square)

                C00 = plane(hp, "C00")
                C01 = plane(hp, "C01")
                C02 = plane(hp, "C02")
                C11 = plane(hp, "C11")
                C12 = plane(hp, "C12")
                C22 = plane(hp, "C22")

                t1 = plane(tmpp, "tmp")
                v.tensor_mul(t1[:], e[:], h[:])
                v.tensor_sub(C00[:], t1[:], f2sq[:])

                t2 = plane(tmpp, "tmp")
                t3 = plane(tmpp, "tmp")
                v.tensor_mul(t2[:], SE[:], h[:])
                v.tensor_mul(t3[:], f2[:], q2[:])
                v.tensor_sub(C01[:], t2[:], t3[:])

                t4 = plane(tmpp, "tmp")
                t5 = plane(tmpp, "tmp")
                v.tensor_mul(t4[:], SE[:], f2[:])
                v.tensor_mul(t5[:], e[:], q2[:])
                v.tensor_sub(C02[:], t4[:], t5[:])

                v.scalar_tensor_tensor(C11[:], h[:], c0, q2sq[:], ALU.mult, ALU.subtract)
                t6 = plane(tmpp, "tmp")
                v.tensor_mul(t6[:], SE[:], q2[:])
                v.scalar_tensor_tensor(C12[:], f2[:], c0, t6[:], ALU.mult, ALU.subtract)
                v.scalar_tensor_tensor(C22[:], e[:], c0, psq[:], ALU.mult, ALU.subtract)

                # ---- det and 1/det ----
                dt1 = plane(tmpp, "tmp")
                dt2 = plane(tmpp, "tmp")
                dt3 = plane(tmpp, "tmp")
                det = plane(hp, "det")
                invdet = plane(hp, "invdet")
                v.tensor_mul(dt1[:], SE[:], C01[:])
                v.scalar_tensor_tensor(dt2[:], C00[:], c0, dt1[:], ALU.mult, ALU.subtract)
                v.tensor_mul(dt3[:], q2[:], C02[:])
                v.tensor_add(det[:], dt2[:], dt3[:])
                v.reciprocal(invdet[:], det[:])

                # ---- gradient ----
                # w_c = beta_c - z_c
                w0 = plane(hp, "w0")
                w1 = plane(hp, "w1")
                w2 = plane(hp, "w2")
                v.tensor_sub(w0[:], betain[:, :, 0], zin[:, :, 0])
                v.tensor_sub(w1[:], betain[:, :, 1], zin[:, :, 1])
                v.tensor_sub(w2[:], betain[:, :, 2], zin[:, :, 2])

                # g0 = A - (B*SE + Sb)/8 + rho*(A + w0)
                g0 = plane(hp, "g0")
                u = plane(tmpp, "tmp")
                ru = plane(tmpp, "tmp")
                v.tensor_add(u[:], Av, w0[:])
                a.mul(ru[:], u[:], rho)
                ta = plane(tmpp, "tmp")
                tb = plane(tmpp, "tmp")
                v.tensor_mul(ta[:], Bv, SE[:])
                v.tensor_add(tb[:], ta[:], Sb[:])
                g0a = plane(tmpp, "tmp")
                v.scalar_tensor_tensor(g0a[:], tb[:], -0.125, ru[:], ALU.mult, ALU.add)
                v.tensor_add(g0[:], g0a[:], Av)

                # g1 = -(A*SE - B*SE2 - SbE)/8 + rho*(B + w1)
                g1 = plane(hp, "g1")
                vb = plane(tmpp, "tmp")
                rv = plane(tmpp, "tmp")
                v.tensor_add(vb[:], Bv, w1[:])
                a.mul(rv[:], vb[:], rho)
                tc1 = plane(tmpp, "tmp")
                tc2 = plane(tmpp, "tmp")
                v.tensor_mul(tc1[:], Av, SE[:])
                v.tensor_mul(tc2[:], Bv, SE2[:])
                tc3 = plane(tmpp, "tmp")
                v.tensor_sub(tc3[:], tc1[:], tc2[:])
                tc4 = plane(tmpp, "tmp")
                v.tensor_sub(tc4[:], tc3[:], SbE[:])
                v.scalar_tensor_tensor(g1[:], tc4[:], -0.125, rv[:], ALU.mult, ALU.add)

                # g2 = B*(A*StE - B*StE2 - SbtE)/8 + rho*(R1 + w2)
                g2 = plane(hp, "g2")
                v2t = plane(tmpp, "tmp")
                rv2 = plane(tmpp, "tmp")
                v.tensor_add(v2t[:], R1v, w2[:])
                a.mul(rv2[:], v2t[:], rho)
                td1 = plane(tmpp, "tmp")
                v.tensor_mul(td1[:], Av, StE[:])
                td2 = plane(tmpp, "tmp")
                v.tensor_sub(td2[:], td1[:], f2[:])
                td3 = plane(tmpp, "tmp")
                v.tensor_sub(td3[:], td2[:], SbtE[:])
                td4 = plane(tmpp, "tmp")
                v.tensor_mul(td4[:], Bv, td3[:])
                v.scalar_tensor_tensor(g2[:], td4[:], 0.125, rv2[:], ALU.mult, ALU.add)

                # ---- d = (C/det) @ g ;  out = x - d ----
                g0s = plane(hp, "g0s")
                g1s = plane(hp, "g1s")
                g2s = plane(hp, "g2s")
                v.tensor_mul(g0s[:], g0[:], invdet[:])
                v.tensor_mul(g1s[:], g1[:], invdet[:])
                v.tensor_mul(g2s[:], g2[:], invdet[:])

                for c, (Ca, Cb, Cc) in enumerate(
                    [(C00, C01, C02), (C01, C11, C12), (C02, C12, C22)]
                ):
                    m0 = plane(tmpp, "tmp")
                    m1 = plane(tmpp, "tmp")
                    m2 = plane(tmpp, "tmp")
                    v.tensor_mul(m0[:], Ca[:], g0s[:])
                    v.tensor_mul(m1[:], Cb[:], g1s[:])
                    v.tensor_mul(m2[:], Cc[:], g2s[:])
                    s0 = plane(tmpp, "tmp")
                    v.tensor_add(s0[:], m0[:], m1[:])
                    dsum = plane(tmpp, "tmp")
                    v.tensor_add(dsum[:], s0[:], m2[:])
                    v.tensor_sub(yout[:, :, c], xin[:, :, c], dsum[:])
                    if chk is not None and c == 2:
                        v.tensor_add(chk[:], chk[:], dsum[:])

            nc.sync.dma_start(yr[:, :, :], yout_g[:])
            if chk is not None:
                nc.sync.dma_start(chkd[:], chk[:])

    _split_excess_waits(nc)
    return nc


def _build5(tau, rho, reps=1, pool_load="none", act_off=True, ring_bufs=3, share_planes=False, sb_adds=False, two_pass=False):
    """v5: three-engine op-split, f32, LDL solve (stable, SPD), one 1024-wide
    chunk per rep.

    ACT: E_q=exp(-tau_q R1), E2_q=exp(-2 tau_q R1) (rings), B^2, SE^2, q2^2.
    Pool (TT-only; TensorScalarPtr is illegal on Pool): unweighted chains
    SE/SE2/SbE, Eb muls, w=beta-z (once), gradient helper muls.
    DVE: weighted STT chains StE/StE2/St2E2/SbtE, Sb reduce, H entries,
    LDL factor+solve (2 reciprocals, rho/2 clamps), ghat assembly, y out.

    ghat = 8*grad so the 1/8 of mean-over-q folds into the final
    y = x - ghat_solve/8 STTs. y overwrites x in SBUF (x dead by then);
    for reps>1 timing builds later reps compute garbage but identical work.
    """
    tau = [float(t) for t in tau]
    rho = float(rho)
    c0 = 8.0 + rho
    i0 = 1.0 / c0
    e8r = 8.0 * rho
    clamp = 0.5 * rho

    nc = bass.Bass()
    xd = nc.declare_dram_parameter("x", [PIX_CORE, NP], F32, isOutput=False)
    zd = nc.declare_dram_parameter("z", [PIX_CORE, NP], F32, isOutput=False)
    betad = nc.declare_dram_parameter("beta", [PIX_CORE, NP], F32, isOutput=False)
    bd = nc.declare_dram_parameter("b", [PIX_CORE, NQ], F32, isOutput=False)
    yd = nc.declare_dram_parameter("y", [PIX_CORE, NP], F32, isOutput=True)

    xr = xd.rearrange("(p f) c -> p f c", p=PARTS)
    zr = zd.rearrange("(p f) c -> p f c", p=PARTS)
    betar = betad.rearrange("(p f) c -> p f c", p=PARTS)
    br = bd.rearrange("(p f) q -> p f q", p=PARTS)
    yr = yd.rearrange("(p f) c -> p f c", p=PARTS)
    chkd = None
    if reps > 1:
        chkd = nc.declare_dram_parameter("chk", [PARTS, 256], F32, isOutput=True)

    v = nc.vector
    a = nc.scalar
    gp = nc.gpsimd
    # g: engine for the unweighted sum chains / Eb muls
    # gh: engine for gradient helper muls and the one-time w chain
    g = gp if pool_load in ("light", "full") else nc.vector
    gh = gp if pool_load == "full" else nc.vector

    F = NFREE  # 1024

    with TileContext(nc) as tc:
        with (
            tc.tile_pool(name="io", bufs=1) as io,
            tc.tile_pool(name="ering", bufs=ring_bufs) as ering,
            tc.tile_pool(name="e2ring", bufs=ring_bufs) as e2ring,
            tc.tile_pool(name="ebring", bufs=ring_bufs) as ebring,
            tc.tile_pool(name="sums", bufs=1) as sums,
            tc.tile_pool(name="hp", bufs=1) as hp,
        ):
            xin = io.tile([PARTS, F, NP], F32, tag="xin", name="xin")
            zin = io.tile([PARTS, F, NP], F32, tag="zin", name="zin")
            betain = io.tile([PARTS, F, NP], F32, tag="betain", name="betain")
            bin_ = io.tile([PARTS, F, NQ], F32, tag="bin", name="bin")
            w = io.tile([PARTS, F, NP], F32, tag="w", name="w")
            nc.sync.dma_start(xin[:], xr[:, :, :])
            nc.sync.dma_start(zin[:], zr[:, :, :])
            nc.sync.dma_start(betain[:], betar[:, :, :])
            nc.sync.dma_start(bin_[:], br[:, :, :])

            chk = None
            if reps > 1:
                chk = io.tile([PARTS, 256], F32, tag="chk", name="chk")
                v.memset(chk[:], 0.0)

            A = xin[:, :, 0]
            Bv = xin[:, :, 1]
            R1 = xin[:, :, 2]

            rho_ap = None
            if act_off:
                rho_t = io.tile([PARTS, 1], F32, tag="rho", name="rho_t")
                nc.gpsimd.memset(rho_t[:], rho)
                rho_ap = rho_t[:]

            # one-time: w = beta - z ; afterwards zin/betain are dead and
            # their storage is reused as LDL scratch planes below.
            gh.tensor_sub(w[:], betain[:], zin[:])
            w0, w1, w2 = w[:, :, 0], w[:, :, 1], w[:, :, 2]
            zf = zin[:].rearrange("p f c -> p (f c)")
            bf = betain[:].rearrange("p f c -> p (f c)")
            i1 = zf[:, 0:F]
            i2 = zf[:, F:2 * F]
            m10 = zf[:, 2 * F:3 * F]
            m20 = bf[:, 0:F]
            t2 = bf[:, F:2 * F]
            t3 = bf[:, 2 * F:3 * F]

            def splane(tag):
                return sums.tile([PARTS, F], F32, tag=tag, name=tag)

            SE, StE, SE2, StE2, St2E2 = (
                splane(n) for n in ("SE", "StE", "SE2", "StE2", "St2E2"))
            Sb, SbE, SbtE = splane("Sb"), splane("SbE"), splane("SbtE")

            def plane(tag):
                return hp.tile([PARTS, F], F32, tag=tag, name=tag)

            q2, f2, e, h = plane("q2"), plane("f2"), plane("e"), plane("h")
            B2, ps, t1 = plane("B2"), plane("ps"), plane("t1")
            tg0, tg1, tg2 = plane("tg0"), plane("tg1"), plane("tg2")
            if share_planes:
                q2s = B2
                g0, g1, g2 = tg0, tg1, tg2
            else:
                q2s = plane("q2s")
                g0, g1, g2 = plane("g0"), plane("g1"), plane("g2")

            for _ in range(reps):
                # ---- ACT: B2 + exp families into rings ----
                a.activation(B2[:], Bv, ACTF.Square)
                E = []
                E2 = []
                Eb = []
                if two_pass:
                    # Pass 1: E-ring consumers only (5 streams: Eb, SbtE,
                    # SbE, StE, SE). Pass 2: E2-ring consumers (3 streams).
                    for q in range(NQ):
                        Eq = ering.tile([PARTS, F], F32, tag="E", name=f"E{q}")
                        a.activation(Eq[:], R1, ACTF.Exp, scale=-tau[q])
                        E.append(Eq)
                        Ebq = ebring.tile([PARTS, F], F32, tag="Eb", name=f"Eb{q}")
                        v.tensor_mul(Ebq[:], Eq[:], bin_[:, :, q])
                        Eb.append(Ebq)
                        tq = tau[q]
                        if q == 0:
                            v.tensor_reduce(Sb[:], bin_[:], mybir.AxisListType.X, ALU.add)
                            if act_off:
                                a.mul(StE[:], Eq[:], tq)
                            else:
                                v.tensor_scalar_mul(StE[:], Eq[:], tq)
                            v.tensor_scalar_mul(SbtE[:], Ebq[:], tq)
                        else:
                            v.scalar_tensor_tensor(StE[:], Eq[:], tq, StE[:], ALU.mult, ALU.add)
                            v.scalar_tensor_tensor(SbtE[:], Ebq[:], tq, SbtE[:], ALU.mult, ALU.add)
                            if q == 1:
                                v.tensor_add(SE[:], E[0][:], E[1][:])
                                v.tensor_add(SbE[:], Eb[0][:], Eb[1][:])
                            else:
                                v.tensor_add(SE[:], SE[:], Eq[:])
                                v.tensor_add(SbE[:], SbE[:], Ebq[:])
                    for q in range(NQ):
                        E2q = e2ring.tile([PARTS, F], F32, tag="E2", name=f"E2{q}")
                        a.activation(E2q[:], R1, ACTF.Exp, scale=-2.0 * tau[q])
                        E2.append(E2q)
                        tq = tau[q]
                        if q == 0:
                            if act_off:
                                a.mul(StE2[:], E2q[:], tq)
                                a.mul(St2E2[:], E2q[:], tq * tq)
                            else:
                                v.tensor_scalar_mul(StE2[:], E2q[:], tq)
                                v.tensor_scalar_mul(St2E2[:], E2q[:], tq * tq)
                        else:
                            v.scalar_tensor_tensor(StE2[:], E2q[:], tq, StE2[:], ALU.mult, ALU.add)
                            v.scalar_tensor_tensor(St2E2[:], E2q[:], tq * tq, St2E2[:], ALU.mult, ALU.add)
                            if q == 1:
                                v.tensor_add(SE2[:], E2[0][:], E2[1][:])
                            else:
                                v.tensor_add(SE2[:], SE2[:], E2q[:])
                elif True:
                    pass
                if two_pass:
                    pass
                else:
                  for q in range(NQ):
                    Eq = ering.tile([PARTS, F], F32, tag="E", name=f"E{q}")
                    a.activation(Eq[:], R1, ACTF.Exp, scale=-tau[q])
                    E.append(Eq)
                    E2q = e2ring.tile([PARTS, F], F32, tag="E2", name=f"E2{q}")
                    a.activation(E2q[:], R1, ACTF.Exp, scale=-2.0 * tau[q])
                    E2.append(E2q)

                    # ---- Pool: unweighted chains, q-interleaved ----
                    Ebq = ebring.tile([PARTS, F], F32, tag="Eb", name=f"Eb{q}")
                    g.tensor_mul(Ebq[:], Eq[:], bin_[:, :, q])
                    Eb.append(Ebq)
                    if q == 1:
                        g.tensor_add(SE[:], E[0][:], E[1][:])
                        g.tensor_add(SE2[:], E2[0][:], E2[1][:])
                        g.tensor_add(SbE[:], Eb[0][:], Eb[1][:])
                        if sb_adds:
                            v.tensor_add(Sb[:], bin_[:, :, 0], bin_[:, :, 1])
                    elif q > 1:
                        g.tensor_add(SE[:], SE[:], Eq[:])
                        g.tensor_add(SE2[:], SE2[:], E2q[:])
                        g.tensor_add(SbE[:], SbE[:], Ebq[:])
                        if sb_adds:
                            v.tensor_add(Sb[:], Sb[:], bin_[:, :, q])

                    # ---- DVE: weighted STT chains, q-interleaved ----
                    tq = tau[q]
                    if q == 0:
                        if not sb_adds:
                            v.tensor_reduce(Sb[:], bin_[:], mybir.AxisListType.X, ALU.add)
                        if act_off:
                            a.mul(StE[:], Eq[:], tq)
                            a.mul(StE2[:], E2q[:], tq)
                            a.mul(St2E2[:], E2q[:], tq * tq)
                        else:
                            v.tensor_scalar_mul(StE[:], Eq[:], tq)
                            v.tensor_scalar_mul(StE2[:], E2q[:], tq)
                            v.tensor_scalar_mul(St2E2[:], E2q[:], tq * tq)
                        # SbtE init reads DVE-produced Eb0; keep it on DVE so
                        # the ACT exp stream never blocks on DVE.
                        v.tensor_scalar_mul(SbtE[:], Ebq[:], tq)
                    else:
                        v.scalar_tensor_tensor(StE[:], Eq[:], tq, StE[:], ALU.mult, ALU.add)
                        v.scalar_tensor_tensor(StE2[:], E2q[:], tq, StE2[:], ALU.mult, ALU.add)
                        v.scalar_tensor_tensor(St2E2[:], E2q[:], tq * tq, St2E2[:], ALU.mult, ALU.add)
                        v.scalar_tensor_tensor(SbtE[:], Ebq[:], tq, SbtE[:], ALU.mult, ALU.add)

                # ---- Pool: gradient helper muls ----
                gh.tensor_mul(tg0[:], Bv, SE[:])
                gh.tensor_add(tg0[:], tg0[:], Sb[:])
                gh.tensor_mul(tg1[:], A, SE[:])
                gh.tensor_mul(t3, Bv, SE2[:])
                gh.tensor_sub(tg1[:], tg1[:], t3)
                gh.tensor_sub(tg1[:], tg1[:], SbE[:])
                gh.tensor_mul(tg2[:], A, StE[:])

                # ---- ACT: squares of DVE/Pool-produced planes ----
                a.activation(ps[:], SE[:], ACTF.Square)

                # ---- DVE: H entries ----
                v.tensor_mul(q2[:], Bv, StE[:])
                v.tensor_mul(f2[:], Bv, StE2[:])
                if act_off:
                    a.add(e[:], SE2[:], rho_ap)
                else:
                    v.tensor_scalar_add(e[:], SE2[:], rho)
                v.tensor_mul(h[:], B2[:], St2E2[:])
                if act_off:
                    a.add(h[:], h[:], rho_ap)
                else:
                    v.tensor_scalar_add(h[:], h[:], rho)

                a.activation(q2s[:], q2[:], ACTF.Square)

                # ---- DVE: LDL factorization ----
                # d1 = e - i0*ps   (clamped at rho/2)
                v.scalar_tensor_tensor(e[:], ps[:], -i0, e[:], ALU.mult, ALU.add)
                v.tensor_scalar(e[:], e[:], clamp, None, ALU.max)
                d1 = e
                v.tensor_mul(t1[:], SE[:], q2[:])
                # u12 = i0*(SE*q2) - f2
                v.scalar_tensor_tensor(t1[:], t1[:], i0, f2[:], ALU.mult, ALU.subtract)
                u12 = t1
                v.reciprocal(i1, d1[:])
                v.tensor_mul(t2, u12[:], u12[:])       # u12^2 (before overwrite)
                v.tensor_mul(u12[:], u12[:], i1)       # l21 = u12 * i1 (in place)
                l21 = u12
                # d2 = h - i0*q2s - u12^2*i1, clamped at rho/2
                v.scalar_tensor_tensor(h[:], q2s[:], -i0, h[:], ALU.mult, ALU.add)
                v.tensor_mul(t2, t2, i1)
                v.tensor_sub(h[:], h[:], t2)
                d2 = h
                v.tensor_scalar(d2[:], d2[:], clamp, None, ALU.max)
                v.reciprocal(i2, d2[:])
                v.tensor_scalar_mul(m10, SE[:], i0)
                v.tensor_scalar_mul(m20, q2[:], i0)

                # ---- DVE: ghat assembly ----
                v.scalar_tensor_tensor(g0[:], w0, e8r, tg0[:], ALU.mult, ALU.subtract)
                v.scalar_tensor_tensor(g0[:], A, 8.0 + e8r, g0[:], ALU.mult, ALU.add)
                v.scalar_tensor_tensor(g1[:], w1, e8r, tg1[:], ALU.mult, ALU.subtract)
                v.scalar_tensor_tensor(g1[:], Bv, e8r, g1[:], ALU.mult, ALU.add)
                v.tensor_sub(tg2[:], tg2[:], f2[:])
                v.tensor_sub(tg2[:], tg2[:], SbtE[:])
                v.tensor_mul(tg2[:], Bv, tg2[:])
                v.scalar_tensor_tensor(g2[:], w2, e8r, tg2[:], ALU.mult, ALU.add)
                v.scalar_tensor_tensor(g2[:], R1, e8r, g2[:], ALU.mult, ALU.add)

                # ---- DVE: LDL solve ----
                v.tensor_mul(t2, m10, g0[:])
                v.tensor_add(g1[:], g1[:], t2)          # y1
                v.tensor_mul(t2, m20, g0[:])
                v.tensor_sub(g2[:], g2[:], t2)          # y2 partial
                v.tensor_mul(t2, l21[:], g1[:])
                v.tensor_sub(g2[:], g2[:], t2)          # y2
                if act_off:
                    a.mul(g0[:], g0[:], i0)             # z0
                else:
                    v.tensor_scalar_mul(g0[:], g0[:], i0)   # z0
                v.tensor_mul(g1[:], g1[:], i1)          # z1
                v.tensor_mul(g2[:], g2[:], i2)          # z2
                v.tensor_mul(t2, l21[:], g2[:])
                v.tensor_sub(g1[:], g1[:], t2)          # d1_
                v.tensor_mul(t2, m10, g1[:])
                v.tensor_add(g0[:], g0[:], t2)
                v.tensor_mul(t2, m20, g2[:])
                v.tensor_sub(g0[:], g0[:], t2)          # d0_

                # ---- y = x - d/8 (in place over x) ----
                v.scalar_tensor_tensor(xin[:, :, 0], g0[:], -0.125, A, ALU.mult, ALU.add)
                v.scalar_tensor_tensor(xin[:, :, 1], g1[:], -0.125, Bv, ALU.mult, ALU.add)
                v.scalar_tensor_tensor(xin[:, :, 2], g2[:], -0.125, R1, ALU.mult, ALU.add)

                if chk is not None:
                    xf = xin[:].rearrange("p f c -> p (f c)")
                    v.tensor_add(chk[:], chk[:], xf[:, 0:256])

            nc.sync.dma_start(yr[:, :, :], xin[:])
            if chk is not None:
                nc.sync.dma_start(chkd[:], chk[:])

    _split_excess_waits(nc)
    return nc




def _build6(tau, rho, reps=1, qring=2):
    """v6: like v5 (all-DVE + ACT offload) but with packed q-rings.

    Per q one [128, F, 4] ring tile Q holds (E, E2, Eb, TE2=tau*E2); the four
    weighted sums (StE, StE2, SbtE, St2E2) accumulate with ONE STT per q into
    a packed SS[128,F,4], and the three unweighted sums (SE, SE2, SbE) with
    one add per q into SU[128,F,3]. Downstream reads use strided slices.
    TE2 comes from ACT as exp(-2 tau R1 + ln tau) via per-q bias tiles.
    """
    tau = [float(t) for t in tau]
    rho = float(rho)
    c0 = 8.0 + rho
    i0 = 1.0 / c0
    e8r = 8.0 * rho
    clamp = 0.5 * rho

    nc = bass.Bass()
    xd = nc.declare_dram_parameter("x", [PIX_CORE, NP], F32, isOutput=False)
    zd = nc.declare_dram_parameter("z", [PIX_CORE, NP], F32, isOutput=False)
    betad = nc.declare_dram_parameter("beta", [PIX_CORE, NP], F32, isOutput=False)
    bd = nc.declare_dram_parameter("b", [PIX_CORE, NQ], F32, isOutput=False)
    yd = nc.declare_dram_parameter("y", [PIX_CORE, NP], F32, isOutput=True)

    xr = xd.rearrange("(p f) c -> p f c", p=PARTS)
    zr = zd.rearrange("(p f) c -> p f c", p=PARTS)
    betar = betad.rearrange("(p f) c -> p f c", p=PARTS)
    br = bd.rearrange("(p f) q -> p f q", p=PARTS)
    yr = yd.rearrange("(p f) c -> p f c", p=PARTS)
    chkd = None
    if reps > 1:
        chkd = nc.declare_dram_parameter("chk", [PARTS, 256], F32, isOutput=True)

    v = nc.vector
    a = nc.scalar
    F = NFREE

    with TileContext(nc) as tc:
        with (
            tc.tile_pool(name="io", bufs=1) as io,
            tc.tile_pool(name="qring", bufs=qring) as qpool,
            tc.tile_pool(name="acc", bufs=1) as acc,
            tc.tile_pool(name="hp", bufs=1) as hp,
        ):
            xin = io.tile([PARTS, F, NP], F32, tag="xin", name="xin")
            zin = io.tile([PARTS, F, NP], F32, tag="zin", name="zin")
            betain = io.tile([PARTS, F, NP], F32, tag="betain", name="betain")
            bin_ = io.tile([PARTS, F, NQ], F32, tag="bin", name="bin")
            w = io.tile([PARTS, F, NP], F32, tag="w", name="w")
            nc.sync.dma_start(xin[:], xr[:, :, :])
            nc.sync.dma_start(zin[:], zr[:, :, :])
            nc.sync.dma_start(betain[:], betar[:, :, :])
            nc.sync.dma_start(bin_[:], br[:, :, :])

            chk = None
            if reps > 1:
                chk = io.tile([PARTS, 256], F32, tag="chk", name="chk")
                v.memset(chk[:], 0.0)

            A = xin[:, :, 0]
            Bv = xin[:, :, 1]
            R1 = xin[:, :, 2]

            rho_t = io.tile([PARTS, 1], F32, tag="rho", name="rho_t")
            nc.gpsimd.memset(rho_t[:], rho)
            rho_ap = rho_t[:]
            lnt = []
            for q in range(NQ):
                t_ = io.tile([PARTS, 1], F32, tag=f"lnt{q}", name=f"lnt{q}")
                nc.gpsimd.memset(t_[:], float(np.log(tau[q])))
                lnt.append(t_)

            # one-time: w = beta - z; zin/betain storage becomes LDL scratch
            v.tensor_sub(w[:], betain[:], zin[:])
            w0, w1, w2 = w[:, :, 0], w[:, :, 1], w[:, :, 2]
            zf = zin[:].rearrange("p f c -> p (f c)")
            bf = betain[:].rearrange("p f c -> p (f c)")
            i1 = zf[:, 0:F]
            i2 = zf[:, F:2 * F]
            m10 = zf[:, 2 * F:3 * F]
            m20 = bf[:, 0:F]
            t2 = bf[:, F:2 * F]
            t3 = bf[:, 2 * F:3 * F]

            SU = acc.tile([PARTS, F, 3], F32, tag="SU", name="SU")
            SS = acc.tile([PARTS, F, 4], F32, tag="SS", name="SS")
            Sb = acc.tile([PARTS, F], F32, tag="Sb", name="Sb")

            def plane(tag):
                return hp.tile([PARTS, F], F32, tag=tag, name=tag)

            q2, f2, e, h = plane("q2"), plane("f2"), plane("e"), plane("h")
            B2, ps, q2s, t1 = plane("B2"), plane("ps"), plane("q2s"), plane("t1")
            tg0, tg1, tg2 = plane("tg0"), plane("tg1"), plane("tg2")
            g0, g1, g2 = plane("g0"), plane("g1"), plane("g2")

            for _ in range(reps):
                a.activation(B2[:], Bv, ACTF.Square)
                Q = []
                for q in range(NQ):
                    Qq = qpool.tile([PARTS, F, 4], F32, tag="Q", name=f"Q{q}")
                    a.activation(Qq[:, :, 0], R1, ACTF.Exp, scale=-tau[q])
                    a.activation(Qq[:, :, 1], R1, ACTF.Exp, scale=-2.0 * tau[q])
                    a.activation(Qq[:, :, 3], R1, ACTF.Exp, scale=-2.0 * tau[q], bias=lnt[q][:])
                    Q.append(Qq)

                    # DVE
                    v.tensor_mul(Qq[:, :, 2], Qq[:, :, 0], bin_[:, :, q])
                    tq = tau[q]
                    if q == 0:
                        v.tensor_reduce(Sb[:], bin_[:], mybir.AxisListType.X, ALU.add)
                        v.tensor_scalar_mul(SS[:], Qq[:], tq)
                    else:
                        v.scalar_tensor_tensor(SS[:], Qq[:], tq, SS[:], ALU.mult, ALU.add)
                        if q == 1:
                            v.tensor_add(SU[:], Q[0][:, :, 0:3], Q[1][:, :, 0:3])
                        else:
                            v.tensor_add(SU[:], SU[:], Qq[:, :, 0:3])

                SE = SU[:, :, 0]
                SE2 = SU[:, :, 1]
                SbE = SU[:, :, 2]
                StE = SS[:, :, 0]
                StE2 = SS[:, :, 1]
                SbtE = SS[:, :, 2]
                St2E2 = SS[:, :, 3]

                # gradient helper muls (DVE)
                v.tensor_mul(tg0[:], Bv, SE)
                v.tensor_add(tg0[:], tg0[:], Sb[:])
                v.tensor_mul(tg1[:], A, SE)
                v.tensor_mul(t3, Bv, SE2)
                v.tensor_sub(tg1[:], tg1[:], t3)
                v.tensor_sub(tg1[:], tg1[:], SbE)
                v.tensor_mul(tg2[:], A, StE)

                a.activation(ps[:], SE, ACTF.Square)

                # H entries
                v.tensor_mul(q2[:], Bv, StE)
                v.tensor_mul(f2[:], Bv, StE2)
                a.add(e[:], SE2, rho_ap)
                v.tensor_mul(h[:], B2[:], St2E2)
                a.add(h[:], h[:], rho_ap)

                a.activation(q2s[:], q2[:], ACTF.Square)

                # LDL factorization
                v.scalar_tensor_tensor(e[:], ps[:], -i0, e[:], ALU.mult, ALU.add)
                v.tensor_scalar(e[:], e[:], clamp, None, ALU.max)
                d1 = e
                v.tensor_mul(t1[:], SE, q2[:])
                v.scalar_tensor_tensor(t1[:], t1[:], i0, f2[:], ALU.mult, ALU.subtract)
                u12 = t1
                v.reciprocal(i1, d1[:])
                v.tensor_mul(t2, u12[:], u12[:])
                v.tensor_mul(u12[:], u12[:], i1)
                l21 = u12
                v.scalar_tensor_tensor(h[:], q2s[:], -i0, h[:], ALU.mult, ALU.add)
                v.tensor_mul(t2, t2, i1)
                v.tensor_sub(h[:], h[:], t2)
                d2 = h
                v.tensor_scalar(d2[:], d2[:], clamp, None, ALU.max)
                v.reciprocal(i2, d2[:])
                v.tensor_scalar_mul(m10, SE, i0)
                v.tensor_scalar_mul(m20, q2[:], i0)

                # ghat
                v.scalar_tensor_tensor(g0[:], w0, e8r, tg0[:], ALU.mult, ALU.subtract)
                v.scalar_tensor_tensor(g0[:], A, 8.0 + e8r, g0[:], ALU.mult, ALU.add)
                v.scalar_tensor_tensor(g1[:], w1, e8r, tg1[:], ALU.mult, ALU.subtract)
                v.scalar_tensor_tensor(g1[:], Bv, e8r, g1[:], ALU.mult, ALU.add)
                v.tensor_sub(tg2[:], tg2[:], f2[:])
                v.tensor_sub(tg2[:], tg2[:], SbtE)
                v.tensor_mul(tg2[:], Bv, tg2[:])
                v.scalar_tensor_tensor(g2[:], w2, e8r, tg2[:], ALU.mult, ALU.add)
                v.scalar_tensor_tensor(g2[:], R1, e8r, g2[:], ALU.mult, ALU.add)

                # solve
                v.tensor_mul(t2, m10, g0[:])
                v.tensor_add(g1[:], g1[:], t2)
                v.tensor_mul(t2, m20, g0[:])
                v.tensor_sub(g2[:], g2[:], t2)
                v.tensor_mul(t2, l21[:], g1[:])
                v.tensor_sub(g2[:], g2[:], t2)
                a.mul(g0[:], g0[:], i0)
                v.tensor_mul(g1[:], g1[:], i1)
                v.tensor_mul(g2[:], g2[:], i2)
                v.tensor_mul(t2, l21[:], g2[:])
                v.tensor_sub(g1[:], g1[:], t2)
                v.tensor_mul(t2, m10, g1[:])
                v.tensor_add(g0[:], g0[:], t2)
                v.tensor_mul(t2, m20, g2[:])
                v.tensor_sub(g0[:], g0[:], t2)

                v.scalar_tensor_tensor(xin[:, :, 0], g0[:], -0.125, A, ALU.mult, ALU.add)
                v.scalar_tensor_tensor(xin[:, :, 1], g1[:], -0.125, Bv, ALU.mult, ALU.add)
                v.scalar_tensor_tensor(xin[:, :, 2], g2[:], -0.125, R1, ALU.mult, ALU.add)

                if chk is not None:
                    xf = xin[:].rearrange("p f c -> p (f c)")
                    v.tensor_add(chk[:], chk[:], xf[:, 0:256])

            nc.sync.dma_start(yr[:, :, :], xin[:])
            if chk is not None:
                nc.sync.dma_start(chkd[:], chk[:])

    _split_excess_waits(nc)
    return nc



def _build7(tau, rho, reps=1, nsplit=2, pool_on=True):
    """v7: latency-aware three-engine split.

    Facts this build encodes (measured on this system):
    - DVE is in-order; back-to-back RAW-dependent ops cost ~2.4us each, but
      >=2 independent interleaved streams pipeline at ~1.25ns/elem.
    - ACT sustains ~1.08ns/elem on unary ops (exp/square/copy-scale).
    - Pool sustains ~2.2-2.5ns/elem on tensor-tensor ops; it must only ever
      wait on FAST producers (ACT/DVE), never sit on DVE's critical path.

    Split: Pool runs the unweighted sum chains (SE/SE2/SbE), the gradient
    helper muls and the one-time w=beta-z. DVE runs Eb muls + the four
    weighted STT chains (5 interleaved streams), then the whole post phase
    (H/LDL/solve/ghat/y) pixel-split into `nsplit` interleaved half-streams
    so consecutive DVE instructions are independent. ACT: exps, squares,
    +rho biases, chain inits, z0.
    """
    tau = [float(t) for t in tau]
    rho = float(rho)
    c0 = 8.0 + rho
    i0 = 1.0 / c0
    e8r = 8.0 * rho
    clamp = 0.5 * rho

    nc = bass.Bass()
    xd = nc.declare_dram_parameter("x", [PIX_CORE, NP], F32, isOutput=False)
    zd = nc.declare_dram_parameter("z", [PIX_CORE, NP], F32, isOutput=False)
    betad = nc.declare_dram_parameter("beta", [PIX_CORE, NP], F32, isOutput=False)
    bd = nc.declare_dram_parameter("b", [PIX_CORE, NQ], F32, isOutput=False)
    yd = nc.declare_dram_parameter("y", [PIX_CORE, NP], F32, isOutput=True)

    xr = xd.rearrange("(p f) c -> p f c", p=PARTS)
    zr = zd.rearrange("(p f) c -> p f c", p=PARTS)
    betar = betad.rearrange("(p f) c -> p f c", p=PARTS)
    br = bd.rearrange("(p f) q -> p f q", p=PARTS)
    yr = yd.rearrange("(p f) c -> p f c", p=PARTS)
    chkd = None
    if reps > 1:
        chkd = nc.declare_dram_parameter("chk", [PARTS, 256], F32, isOutput=True)

    v = nc.vector
    a = nc.scalar
    g = nc.gpsimd if pool_on else nc.vector
    F = NFREE

    with TileContext(nc) as tc:
        with (
            tc.tile_pool(name="io", bufs=1) as io,
            tc.tile_pool(name="ering", bufs=ring_bufs) as ering,
            tc.tile_pool(name="e2ring", bufs=ring_bufs) as e2ring,
            tc.tile_pool(name="ebring", bufs=ring_bufs) as ebring,
            tc.tile_pool(name="sums", bufs=1) as sums,
            tc.tile_pool(name="hp", bufs=1) as hp,
        ):
            xin = io.tile([PARTS, F, NP], F32, tag="xin", name="xin")
            zin = io.tile([PARTS, F, NP], F32, tag="zin", name="zin")
            betain = io.tile([PARTS, F, NP], F32, tag="betain", name="betain")
            bin_ = io.tile([PARTS, F, NQ], F32, tag="bin", name="bin")
            w = io.tile([PARTS, F, NP], F32, tag="w", name="w")
            nc.sync.dma_start(xin[:], xr[:, :, :])
            nc.sync.dma_start(zin[:], zr[:, :, :])
            nc.sync.dma_start(betain[:], betar[:, :, :])
            nc.sync.dma_start(bin_[:], br[:, :, :])

            chk = None
            if reps > 1:
                chk = io.tile([PARTS, 256], F32, tag="chk", name="chk")
                v.memset(chk[:], 0.0)

            A = xin[:, :, 0]
            Bv = xin[:, :, 1]
            R1 = xin[:, :, 2]

            rho_t = io.tile([PARTS, 1], F32, tag="rho", name="rho_t")
            nc.gpsimd.memset(rho_t[:], rho)
            rho_ap = rho_t[:]

            g.tensor_sub(w[:], betain[:], zin[:])
            zf = zin[:].rearrange("p f c -> p (f c)")
            bf = betain[:].rearrange("p f c -> p (f c)")
            i1 = zf[:, 0:F]
            i2 = zf[:, F:2 * F]
            t2a = zf[:, 2 * F:3 * F]
            t2b = bf[:, 0:F]
            t3 = bf[:, F:2 * F]

            def splane(tag):
                return sums.tile([PARTS, F], F32, tag=tag, name=tag)

            SE, StE, SE2, StE2, St2E2 = (
                splane(n) for n in ("SE", "StE", "SE2", "StE2", "St2E2"))
            Sb, SbE, SbtE = splane("Sb"), splane("SbE"), splane("SbtE")

            def plane(tag):
                return hp.tile([PARTS, F], F32, tag=tag, name=tag)

            q2, f2, e, h = plane("q2"), plane("f2"), plane("e"), plane("h")
            B2, ps, q2s, t1 = plane("B2"), plane("ps"), plane("q2s"), plane("t1")
            tg0, tg1, tg2 = plane("tg0"), plane("tg1"), plane("tg2")
            g0, g1, g2 = plane("g0"), plane("g1"), plane("g2")

            for _ in range(reps):
                # ---- ACT: B2 + exp rings ----
                a.activation(B2[:], Bv, ACTF.Square)
                E = []
                E2 = []
                Eb = []
                for q in range(NQ):
                    Eq = ering.tile([PARTS, F], F32, tag="E", name=f"E{q}")
                    a.activation(Eq[:], R1, ACTF.Exp, scale=-tau[q])
                    E.append(Eq)
                    E2q = e2ring.tile([PARTS, F], F32, tag="E2", name=f"E2{q}")
                    a.activation(E2q[:], R1, ACTF.Exp, scale=-2.0 * tau[q])
                    E2.append(E2q)

                    # DVE: Eb mul + weighted chains (5 interleaved streams)
                    Ebq = ebring.tile([PARTS, F], F32, tag="Eb", name=f"Eb{q}")
                    v.tensor_mul(Ebq[:], Eq[:], bin_[:, :, q])
                    Eb.append(Ebq)
                    tq = tau[q]
                    if q == 0:
                        if sb_pool:
                            g.tensor_add(Sb[:], bin_[:, :, 0], bin_[:, :, 1])
                            for qq in range(2, NQ):
                                g.tensor_add(Sb[:], Sb[:], bin_[:, :, qq])
                        else:
                            v.tensor_reduce(Sb[:], bin_[:], mybir.AxisListType.X, ALU.add)
                        a.mul(StE[:], Eq[:], tq)
                        a.mul(StE2[:], E2q[:], tq)
                        a.mul(St2E2[:], E2q[:], tq * tq)
                        v.tensor_scalar_mul(SbtE[:], Ebq[:], tq)
                    else:
                        v.scalar_tensor_tensor(StE[:], Eq[:], tq, StE[:], ALU.mult, ALU.add)
                        v.scalar_tensor_tensor(StE2[:], E2q[:], tq, StE2[:], ALU.mult, ALU.add)
                        v.scalar_tensor_tensor(St2E2[:], E2q[:], tq * tq, St2E2[:], ALU.mult, ALU.add)
                        v.scalar_tensor_tensor(SbtE[:], Ebq[:], tq, SbtE[:], ALU.mult, ALU.add)

                    # Pool: unweighted chains (3 interleaved streams); only
                    # ever waits on ACT (E/E2) or DVE's Eb (produced early)
                    if q == 1:
                        g.tensor_add(SE[:], E[0][:], E[1][:])
                        g.tensor_add(SE2[:], E2[0][:], E2[1][:])
                        g.tensor_add(SbE[:], Eb[0][:], Eb[1][:])
                    elif q > 1:
                        g.tensor_add(SE[:], SE[:], Eq[:])
                        g.tensor_add(SE2[:], SE2[:], E2q[:])
                        g.tensor_add(SbE[:], SbE[:], Ebq[:])

                # ---- Pool: gradient helper muls (late consumers on DVE) ----
                g.tensor_mul(tg0[:], Bv, SE[:])
                g.tensor_add(tg0[:], tg0[:], Sb[:])
                g.tensor_mul(tg1[:], A, SE[:])
                g.tensor_mul(t3, Bv, SE2[:])
                g.tensor_sub(tg1[:], tg1[:], t3)
                g.tensor_sub(tg1[:], tg1[:], SbE[:])
                g.tensor_mul(tg2[:], A, StE[:])

                # ---- ACT: squares / biases ----
                a.activation(ps[:], SE[:], ACTF.Square)
                a.add(e[:], SE2[:], rho_ap)

                # ---- DVE post phase, pixel-split into nsplit streams ----
                W = F // nsplit
                sls = [slice(k * W, (k + 1) * W) for k in range(nsplit)]

                def S(op, *tensors, **kw):
                    """emit op once per pixel-slice, interleaved"""
                    for sl in sls:
                        args = [t.__getitem__((slice(None), sl)) if isinstance(t, tuple) is False else t for t in tensors]
                        op(*args, **kw)

                # q2, f2 on DVE (needs StE/StE2)
                for sl in sls:
                    v.tensor_mul(q2[:, sl], Bv[:, sl], StE[:, sl])
                    v.tensor_mul(f2[:, sl], Bv[:, sl], StE2[:, sl])
                a.activation(q2s[:], q2[:], ACTF.Square)
                for sl in sls:
                    v.tensor_mul(h[:, sl], B2[:, sl], St2E2[:, sl])
                a.add(h[:], h[:], rho_ap)

                # LDL factor + solve + ghat, interleaved over slices
                def emit(fn):
                    for sl in sls:
                        fn(sl)

                emit(lambda sl: v.scalar_tensor_tensor(e[:, sl], ps[:, sl], -i0, e[:, sl], ALU.mult, ALU.add))
                emit(lambda sl: v.tensor_scalar(e[:, sl], e[:, sl], clamp, None, ALU.max))
                d1 = e
                emit(lambda sl: v.tensor_mul(t1[:, sl], SE[:, sl], q2[:, sl]))
                emit(lambda sl: v.scalar_tensor_tensor(t1[:, sl], t1[:, sl], i0, f2[:, sl], ALU.mult, ALU.subtract))
                u12 = t1
                emit(lambda sl: v.reciprocal(i1[:, sl], d1[:, sl]))
                emit(lambda sl: v.tensor_mul(t2a[:, sl], u12[:, sl], u12[:, sl]))
                emit(lambda sl: v.tensor_mul(u12[:, sl], u12[:, sl], i1[:, sl]))
                l21 = u12
                emit(lambda sl: v.scalar_tensor_tensor(h[:, sl], q2s[:, sl], -i0, h[:, sl], ALU.mult, ALU.add))
                emit(lambda sl: v.tensor_mul(t2a[:, sl], t2a[:, sl], i1[:, sl]))
                emit(lambda sl: v.tensor_sub(h[:, sl], h[:, sl], t2a[:, sl]))
                d2 = h
                emit(lambda sl: v.tensor_scalar(d2[:, sl], d2[:, sl], clamp, None, ALU.max))
                emit(lambda sl: v.reciprocal(i2[:, sl], d2[:, sl]))

                # ghat (interleaves with LDL tail via slice alternation)
                emit(lambda sl: v.scalar_tensor_tensor(g0[:, sl], w[:, sl, 0], e8r, tg0[:, sl], ALU.mult, ALU.subtract))
                emit(lambda sl: v.scalar_tensor_tensor(g0[:, sl], A[:, sl], 8.0 + e8r, g0[:, sl], ALU.mult, ALU.add))
                emit(lambda sl: v.scalar_tensor_tensor(g1[:, sl], w[:, sl, 1], e8r, tg1[:, sl], ALU.mult, ALU.subtract))
                emit(lambda sl: v.scalar_tensor_tensor(g1[:, sl], Bv[:, sl], e8r, g1[:, sl], ALU.mult, ALU.add))
                emit(lambda sl: v.tensor_sub(tg2[:, sl], tg2[:, sl], f2[:, sl]))
                emit(lambda sl: v.tensor_sub(tg2[:, sl], tg2[:, sl], SbtE[:, sl]))
                emit(lambda sl: v.tensor_mul(tg2[:, sl], Bv[:, sl], tg2[:, sl]))
                emit(lambda sl: v.scalar_tensor_tensor(g2[:, sl], w[:, sl, 2], e8r, tg2[:, sl], ALU.mult, ALU.add))
                emit(lambda sl: v.scalar_tensor_tensor(g2[:, sl], R1[:, sl], e8r, g2[:, sl], ALU.mult, ALU.add))

                # z0 on ACT (full width)
                a.mul(g0[:], g0[:], i0)  # now g0 holds z0*c0... wait
                # NOTE: order matters: solve uses z0=g0*i0; we do it via ACT
                # then SE-based reconstruction (m10/m20 eliminated):
                # y1 = g1 + SE*z0 ; y2 = g2 - q2*z0 - l21*y1
                emit(lambda sl: v.tensor_mul(t2a[:, sl], SE[:, sl], g0[:, sl]))
                emit(lambda sl: v.tensor_add(g1[:, sl], g1[:, sl], t2a[:, sl]))
                emit(lambda sl: v.tensor_mul(t2b[:, sl], q2[:, sl], g0[:, sl]))
                emit(lambda sl: v.tensor_sub(g2[:, sl], g2[:, sl], t2b[:, sl]))
                emit(lambda sl: v.tensor_mul(t2a[:, sl], l21[:, sl], g1[:, sl]))
                emit(lambda sl: v.tensor_sub(g2[:, sl], g2[:, sl], t2a[:, sl]))
                emit(lambda sl: v.tensor_mul(g1[:, sl], g1[:, sl], i1[:, sl]))
                emit(lambda sl: v.tensor_mul(g2[:, sl], g2[:, sl], i2[:, sl]))
                emit(lambda sl: v.tensor_mul(t2a[:, sl], l21[:, sl], g2[:, sl]))
                emit(lambda sl: v.tensor_sub(g1[:, sl], g1[:, sl], t2a[:, sl]))
                emit(lambda sl: v.tensor_mul(t2a[:, sl], SE[:, sl], g1[:, sl]))
                emit(lambda sl: v.tensor_mul(t2b[:, sl], q2[:, sl], g2[:, sl]))
                emit(lambda sl: v.tensor_sub(t2a[:, sl], t2a[:, sl], t2b[:, sl]))
                emit(lambda sl: v.scalar_tensor_tensor(g0[:, sl], t2a[:, sl], i0, g0[:, sl], ALU.mult, ALU.add))

                emit(lambda sl: v.scalar_tensor_tensor(xin[:, sl, 0], g0[:, sl], -0.125, A[:, sl], ALU.mult, ALU.add))
                emit(lambda sl: v.scalar_tensor_tensor(xin[:, sl, 1], g1[:, sl], -0.125, Bv[:, sl], ALU.mult, ALU.add))
                emit(lambda sl: v.scalar_tensor_tensor(xin[:, sl, 2], g2[:, sl], -0.125, R1[:, sl], ALU.mult, ALU.add))

                if chk is not None:
                    xf = xin[:].rearrange("p f c -> p (f c)")
                    v.tensor_add(chk[:], chk[:], xf[:, 0:256])

            nc.sync.dma_start(yr[:, :, :], xin[:])
            if chk is not None:
                nc.sync.dma_start(chkd[:], chk[:])

    _split_excess_waits(nc)
    return nc



def _build8(tau, rho, reps=1, pool_tg=True, split_tail=True, phase="full", sb_pool=False, tg2_pool=False):
    """v8: v5 structure with latency-aware post-phase emission order.

    Sums identical to v5 (8 interleaved accumulator streams on DVE, ACT exps
    in rings). Post phase hand-interleaved so consecutive DVE instructions
    are RAW-independent (measured: dependent back-to-back DVE ops cost
    ~2.4us; >=2 independent streams pipeline at ~1.25ns/elem). The final
    forward/back substitution (hard-serial) is pixel-split in two
    interleaved half-streams. Pool gets only decoupled work (w, tg muls).
    """
    tau = [float(t) for t in tau]
    rho = float(rho)
    c0 = 8.0 + rho
    i0 = 1.0 / c0
    e8r = 8.0 * rho
    clamp = 0.5 * rho

    nc = bass.Bass()
    xd = nc.declare_dram_parameter("x", [PIX_CORE, NP], F32, isOutput=False)
    zd = nc.declare_dram_parameter("z", [PIX_CORE, NP], F32, isOutput=False)
    betad = nc.declare_dram_parameter("beta", [PIX_CORE, NP], F32, isOutput=False)
    bd = nc.declare_dram_parameter("b", [PIX_CORE, NQ], F32, isOutput=False)
    yd = nc.declare_dram_parameter("y", [PIX_CORE, NP], F32, isOutput=True)

    xr = xd.rearrange("(p f) c -> p f c", p=PARTS)
    zr = zd.rearrange("(p f) c -> p f c", p=PARTS)
    betar = betad.rearrange("(p f) c -> p f c", p=PARTS)
    br = bd.rearrange("(p f) q -> p f q", p=PARTS)
    yr = yd.rearrange("(p f) c -> p f c", p=PARTS)
    chkd = None
    if reps > 1:
        chkd = nc.declare_dram_parameter("chk", [PARTS, 256], F32, isOutput=True)

    v = nc.vector
    a = nc.scalar
    g = nc.gpsimd if pool_tg else nc.vector
    F = NFREE

    with TileContext(nc) as tc:
        with (
            tc.tile_pool(name="io", bufs=1) as io,
            tc.tile_pool(name="ering", bufs=ring_bufs) as ering,
            tc.tile_pool(name="e2ring", bufs=ring_bufs) as e2ring,
            tc.tile_pool(name="ebring", bufs=ring_bufs) as ebring,
            tc.tile_pool(name="sums", bufs=1) as sums,
            tc.tile_pool(name="hp", bufs=1) as hp,
        ):
            xin = io.tile([PARTS, F, NP], F32, tag="xin", name="xin")
            zin = io.tile([PARTS, F, NP], F32, tag="zin", name="zin")
            betain = io.tile([PARTS, F, NP], F32, tag="betain", name="betain")
            bin_ = io.tile([PARTS, F, NQ], F32, tag="bin", name="bin")
            w = io.tile([PARTS, F, NP], F32, tag="w", name="w")
            nc.sync.dma_start(xin[:], xr[:, :, :])
            nc.sync.dma_start(zin[:], zr[:, :, :])
            nc.sync.dma_start(betain[:], betar[:, :, :])
            nc.sync.dma_start(bin_[:], br[:, :, :])

            chk = None
            if reps > 1:
                chk = io.tile([PARTS, 256], F32, tag="chk", name="chk")
                v.memset(chk[:], 0.0)

            A = xin[:, :, 0]
            Bv = xin[:, :, 1]
            R1 = xin[:, :, 2]

            rho_t = io.tile([PARTS, 1], F32, tag="rho", name="rho_t")
            nc.gpsimd.memset(rho_t[:], rho)
            rho_ap = rho_t[:]

            g.tensor_sub(w[:], betain[:], zin[:])
            w0, w1, w2 = w[:, :, 0], w[:, :, 1], w[:, :, 2]
            zf = zin[:].rearrange("p f c -> p (f c)")
            bf = betain[:].rearrange("p f c -> p (f c)")
            i1 = zf[:, 0:F]
            i2 = zf[:, F:2 * F]
            t2a = zf[:, 2 * F:3 * F]
            t2b = bf[:, 0:F]
            t3 = bf[:, F:2 * F]

            def splane(tag):
                return sums.tile([PARTS, F], F32, tag=tag, name=tag)

            SE, StE, SE2, StE2, St2E2 = (
                splane(n) for n in ("SE", "StE", "SE2", "StE2", "St2E2"))
            Sb, SbE, SbtE = splane("Sb"), splane("SbE"), splane("SbtE")

            def plane(tag):
                return hp.tile([PARTS, F], F32, tag=tag, name=tag)

            q2, f2, e, h = plane("q2"), plane("f2"), plane("e"), plane("h")
            B2, ps, q2s, t1 = plane("B2"), plane("ps"), plane("q2s"), plane("t1")
            tg0, tg1, tg2 = plane("tg0"), plane("tg1"), plane("tg2")
            g0, g1, g2 = plane("g0"), plane("g1"), plane("g2")

            for _ in range(reps):
                # ---- ACT exps + DVE/POOL sum chains (as v5-none) ----
                a.activation(B2[:], Bv, ACTF.Square)
                E = []
                E2 = []
                Eb = []
                for q in range(NQ):
                    Eq = ering.tile([PARTS, F], F32, tag="E", name=f"E{q}")
                    a.activation(Eq[:], R1, ACTF.Exp, scale=-tau[q])
                    E.append(Eq)
                    E2q = e2ring.tile([PARTS, F], F32, tag="E2", name=f"E2{q}")
                    a.activation(E2q[:], R1, ACTF.Exp, scale=-2.0 * tau[q])
                    E2.append(E2q)

                    Ebq = ebring.tile([PARTS, F], F32, tag="Eb", name=f"Eb{q}")
                    v.tensor_mul(Ebq[:], Eq[:], bin_[:, :, q])
                    Eb.append(Ebq)
                    tq = tau[q]
                    if q == 0:
                        if sb_pool:
                            g.tensor_add(Sb[:], bin_[:, :, 0], bin_[:, :, 1])
                            for qq in range(2, NQ):
                                g.tensor_add(Sb[:], Sb[:], bin_[:, :, qq])
                        else:
                            v.tensor_reduce(Sb[:], bin_[:], mybir.AxisListType.X, ALU.add)
                        a.mul(StE[:], Eq[:], tq)
                        a.mul(StE2[:], E2q[:], tq)
                        a.mul(St2E2[:], E2q[:], tq * tq)
                        v.tensor_scalar_mul(SbtE[:], Ebq[:], tq)
                    else:
                        v.scalar_tensor_tensor(StE[:], Eq[:], tq, StE[:], ALU.mult, ALU.add)
                        v.scalar_tensor_tensor(StE2[:], E2q[:], tq, StE2[:], ALU.mult, ALU.add)
                        v.scalar_tensor_tensor(St2E2[:], E2q[:], tq * tq, St2E2[:], ALU.mult, ALU.add)
                        v.scalar_tensor_tensor(SbtE[:], Ebq[:], tq, SbtE[:], ALU.mult, ALU.add)
                        if q == 1:
                            v.tensor_add(SE[:], E[0][:], E[1][:])
                            v.tensor_add(SE2[:], E2[0][:], E2[1][:])
                            v.tensor_add(SbE[:], Eb[0][:], Eb[1][:])
                            if sb_adds:
                                v.tensor_add(Sb[:], bin_[:, :, 0], bin_[:, :, 1])
                        else:
                            v.tensor_add(SE[:], SE[:], Eq[:])
                            v.tensor_add(SE2[:], SE2[:], E2q[:])
                            v.tensor_add(SbE[:], SbE[:], Ebq[:])
                            if sb_adds:
                                v.tensor_add(Sb[:], Sb[:], bin_[:, :, q])

                if phase == "sums":
                    v.scalar_tensor_tensor(xin[:, :, 0], SE[:], -0.125, A, ALU.mult, ALU.add)
                    v.scalar_tensor_tensor(xin[:, :, 1], StE[:], -0.125, Bv, ALU.mult, ALU.add)
                    v.scalar_tensor_tensor(xin[:, :, 2], SbtE[:], -0.125, R1, ALU.mult, ALU.add)
                    if chk is not None:
                        xf = xin[:].rearrange("p f c -> p (f c)")
                        v.tensor_add(chk[:], chk[:], xf[:, 0:256])
                    continue

                # ---- Pool: decoupled gradient helpers ----
                g.tensor_mul(tg0[:], Bv, SE[:])
                g.tensor_add(tg0[:], tg0[:], Sb[:])
                g.tensor_mul(tg1[:], A, SE[:])
                g.tensor_mul(t3, Bv, SE2[:])
                g.tensor_sub(tg1[:], tg1[:], t3)
                g.tensor_sub(tg1[:], tg1[:], SbE[:])
                g.tensor_mul(tg2[:], A, StE[:])

                # ---- ACT: squares + biases (emitted before DVE consumers) ----
                a.activation(ps[:], SE[:], ACTF.Square)
                a.add(e[:], SE2[:], rho_ap)

                # ---- DVE post phase, hand-interleaved streams ----
                v.tensor_mul(q2[:], Bv, StE[:])          # 1  [q2]
                v.tensor_mul(f2[:], Bv, StE2[:])         # 2  [f2]
                v.tensor_mul(h[:], B2[:], St2E2[:])      # 3  [h]
                a.activation(q2s[:], q2[:], ACTF.Square)  # ACT
                a.add(h[:], h[:], rho_ap)                 # ACT
                v.tensor_mul(t1[:], SE[:], q2[:])        # 4  [u12] (q2 dist 3)
                v.scalar_tensor_tensor(e[:], ps[:], -i0, e[:], ALU.mult, ALU.add)   # 5 [d1]
                v.scalar_tensor_tensor(t1[:], t1[:], i0, f2[:], ALU.mult, ALU.subtract)  # 6 [u12]
                v.tensor_scalar(e[:], e[:], clamp, None, ALU.max)   # 7 [d1]
                v.tensor_mul(t2a, t1[:], t1[:])          # 8  [u12^2]
                v.scalar_tensor_tensor(h[:], q2s[:], -i0, h[:], ALU.mult, ALU.add)  # 9 [d2]
                v.reciprocal(i1, e[:])                   # 10 [i1] (d1 dist 3)
                v.scalar_tensor_tensor(g0[:], w0, e8r, tg0[:], ALU.mult, ALU.subtract)  # 11 [g0]
                v.scalar_tensor_tensor(g1[:], w1, e8r, tg1[:], ALU.mult, ALU.subtract)  # 12 [g1]
                v.tensor_mul(t1[:], t1[:], i1)           # 13 [l21] (i1 dist 3)
                v.tensor_mul(t2a, t2a, i1)               # 14 [u12^2*i1]
                v.scalar_tensor_tensor(g0[:], A, 8.0 + e8r, g0[:], ALU.mult, ALU.add)   # 15 [g0]
                v.scalar_tensor_tensor(g1[:], Bv, e8r, g1[:], ALU.mult, ALU.add)        # 16 [g1]
                v.tensor_sub(h[:], h[:], t2a)            # 17 [d2]
                tge = g if tg2_pool else v
                tge.tensor_sub(tg2[:], tg2[:], f2[:])    # 18 [g2 chain]
                v.tensor_scalar(h[:], h[:], clamp, None, ALU.max)  # 19 [d2]
                tge.tensor_sub(tg2[:], tg2[:], SbtE[:])  # 20 [g2]
                a.mul(g0[:], g0[:], i0)                  # ACT: z0 (g0 done @15)
                v.reciprocal(i2, h[:])                   # 21 [i2]
                tge.tensor_mul(tg2[:], Bv, tg2[:])       # 22 [g2]
                v.scalar_tensor_tensor(g2[:], w2, e8r, tg2[:], ALU.mult, ALU.add)  # 23 [g2]
                v.scalar_tensor_tensor(g2[:], R1, e8r, g2[:], ALU.mult, ALU.add)   # 24 [g2]

                # ---- solve tail: hard-serial -> 2-way pixel interleave ----
                l21 = t1
                if split_tail:
                    sls = [slice(0, F // 2), slice(F // 2, F)]
                else:
                    sls = [slice(0, F)]

                def emit(fn):
                    for sl in sls:
                        fn(sl)

                emit(lambda sl: v.tensor_mul(t2a[:, sl], SE[:, sl], g0[:, sl]))
                emit(lambda sl: v.tensor_add(g1[:, sl], g1[:, sl], t2a[:, sl]))
                emit(lambda sl: v.tensor_mul(t2b[:, sl], q2[:, sl], g0[:, sl]))
                emit(lambda sl: v.tensor_sub(g2[:, sl], g2[:, sl], t2b[:, sl]))
                emit(lambda sl: v.tensor_mul(t2a[:, sl], l21[:, sl], g1[:, sl]))
                emit(lambda sl: v.tensor_sub(g2[:, sl], g2[:, sl], t2a[:, sl]))
                emit(lambda sl: v.tensor_mul(g1[:, sl], g1[:, sl], i1[:, sl]))
                emit(lambda sl: v.tensor_mul(g2[:, sl], g2[:, sl], i2[:, sl]))
                emit(lambda sl: v.tensor_mul(t2a[:, sl], l21[:, sl], g2[:, sl]))
                emit(lambda sl: v.tensor_sub(g1[:, sl], g1[:, sl], t2a[:, sl]))
                emit(lambda sl: v.tensor_mul(t2a[:, sl], SE[:, sl], g1[:, sl]))
                emit(lambda sl: v.tensor_mul(t2b[:, sl], q2[:, sl], g2[:, sl]))
                emit(lambda sl: v.tensor_sub(t2a[:, sl], t2a[:, sl], t2b[:, sl]))
                emit(lambda sl: v.scalar_tensor_tensor(g0[:, sl], t2a[:, sl], i0, g0[:, sl], ALU.mult, ALU.add))

                v.scalar_tensor_tensor(xin[:, :, 0], g0[:], -0.125, A, ALU.mult, ALU.add)
                v.scalar_tensor_tensor(xin[:, :, 1], g1[:], -0.125, Bv, ALU.mult, ALU.add)
                v.scalar_tensor_tensor(xin[:, :, 2], g2[:], -0.125, R1, ALU.mult, ALU.add)

                if chk is not None:
                    xf = xin[:].rearrange("p f c -> p (f c)")
                    v.tensor_add(chk[:], chk[:], xf[:, 0:256])

            nc.sync.dma_start(yr[:, :, :], xin[:])
            if chk is not None:
                nc.sync.dma_start(chkd[:], chk[:])

    _split_excess_waits(nc)
    return nc



def _build9(tau, rho, reps=1):
    """v9: v8 + Pool absorbs the SE2/SbE/Sb chains with dedicated rings.

    Ring-pacing rule learned the hard way: an engine that CONSUMES a ring
    gates the producer via WAR once the ring wraps, so a slow consumer
    (Pool) must never share a ring with a fast consumer's critical path.
    ACT therefore writes E2 twice: once into e2ring (DVE's StE2/St2E2
    chains) and once into p2ring (Pool's SE2 chain). Eb is produced by DVE
    and consumed by both DVE (SbtE) and Pool (SbE) from a 4-deep ring;
    the engines run at matched per-q pace so this does not stall.
    Sb is a Pool add-chain over persistent b slices (was a DVE reduce).
    """
    tau = [float(t) for t in tau]
    rho = float(rho)
    c0 = 8.0 + rho
    i0 = 1.0 / c0
    e8r = 8.0 * rho
    clamp = 0.5 * rho

    nc = bass.Bass()
    xd = nc.declare_dram_parameter("x", [PIX_CORE, NP], F32, isOutput=False)
    zd = nc.declare_dram_parameter("z", [PIX_CORE, NP], F32, isOutput=False)
    betad = nc.declare_dram_parameter("beta", [PIX_CORE, NP], F32, isOutput=False)
    bd = nc.declare_dram_parameter("b", [PIX_CORE, NQ], F32, isOutput=False)
    yd = nc.declare_dram_parameter("y", [PIX_CORE, NP], F32, isOutput=True)

    xr = xd.rearrange("(p f) c -> p f c", p=PARTS)
    zr = zd.rearrange("(p f) c -> p f c", p=PARTS)
    betar = betad.rearrange("(p f) c -> p f c", p=PARTS)
    br = bd.rearrange("(p f) q -> p f q", p=PARTS)
    yr = yd.rearrange("(p f) c -> p f c", p=PARTS)
    chkd = None
    if reps > 1:
        chkd = nc.declare_dram_parameter("chk", [PARTS, 252], F32, isOutput=True)

    v = nc.vector
    a = nc.scalar
    g = nc.gpsimd
    F = NFREE

    with TileContext(nc) as tc:
        with (
            tc.tile_pool(name="io", bufs=1) as io,
            tc.tile_pool(name="ering", bufs=3) as ering,
            tc.tile_pool(name="e2ring", bufs=3) as e2ring,
            tc.tile_pool(name="p2ring", bufs=3) as p2ring,
            tc.tile_pool(name="ebring", bufs=4) as ebring,
            tc.tile_pool(name="sums", bufs=1) as sums,
            tc.tile_pool(name="hp", bufs=1) as hp,
        ):
            xin = io.tile([PARTS, F, NP], F32, tag="xin", name="xin")
            zin = io.tile([PARTS, F, NP], F32, tag="zin", name="zin")
            betain = io.tile([PARTS, F, NP], F32, tag="betain", name="betain")
            bin_ = io.tile([PARTS, F, NQ], F32, tag="bin", name="bin")
            w = io.tile([PARTS, F, NP], F32, tag="w", name="w")
            nc.sync.dma_start(xin[:], xr[:, :, :])
            nc.sync.dma_start(zin[:], zr[:, :, :])
            nc.sync.dma_start(betain[:], betar[:, :, :])
            nc.sync.dma_start(bin_[:], br[:, :, :])

            chk = None
            if reps > 1:
                chk = io.tile([PARTS, 252], F32, tag="chk", name="chk")
                v.memset(chk[:], 0.0)

            A = xin[:, :, 0]
            Bv = xin[:, :, 1]
            R1 = xin[:, :, 2]

            rho_t = io.tile([PARTS, 1], F32, tag="rho", name="rho_t")
            nc.gpsimd.memset(rho_t[:], rho)
            rho_ap = rho_t[:]

            g.tensor_sub(w[:], betain[:], zin[:])
            w0, w1, w2 = w[:, :, 0], w[:, :, 1], w[:, :, 2]
            zf = zin[:].rearrange("p f c -> p (f c)")
            bf = betain[:].rearrange("p f c -> p (f c)")
            i1 = zf[:, 0:F]
            i2 = zf[:, F:2 * F]
            t2a = zf[:, 2 * F:3 * F]
            t2b = bf[:, 0:F]
            t3 = bf[:, F:2 * F]
            ps = bf[:, 2 * F:3 * F]

            def splane(tag):
                return sums.tile([PARTS, F], F32, tag=tag, name=tag)

            SE, StE, SE2, StE2, St2E2 = (
                splane(n) for n in ("SE", "StE", "SE2", "StE2", "St2E2"))
            Sb, SbE, SbtE = splane("Sb"), splane("SbE"), splane("SbtE")

            def plane(tag):
                return hp.tile([PARTS, F], F32, tag=tag, name=tag)

            q2, f2, e, h = plane("q2"), plane("f2"), plane("e"), plane("h")
            B2, t1 = plane("B2"), plane("t1")
            g0, g1, g2 = plane("g0"), plane("g1"), plane("g2")
            q2s = B2  # B2 dead after h-mul; q2s born later

            for _ in range(reps):
                a.activation(B2[:], Bv, ACTF.Square)
                E = []
                E2 = []
                P2 = []
                Eb = []
                for q in range(NQ):
                    Eq = ering.tile([PARTS, F], F32, tag="E", name=f"E{q}")
                    a.activation(Eq[:], R1, ACTF.Exp, scale=-tau[q])
                    E.append(Eq)
                    E2q = e2ring.tile([PARTS, F], F32, tag="E2", name=f"E2{q}")
                    a.activation(E2q[:], R1, ACTF.Exp, scale=-2.0 * tau[q])
                    E2.append(E2q)
                    P2q = p2ring.tile([PARTS, F], F32, tag="P2", name=f"P2{q}")
                    a.activation(P2q[:], R1, ACTF.Exp, scale=-2.0 * tau[q])
                    P2.append(P2q)

                    # DVE: Eb mul + weighted chains + SE chain
                    Ebq = ebring.tile([PARTS, F], F32, tag="Eb", name=f"Eb{q}")
                    v.tensor_mul(Ebq[:], Eq[:], bin_[:, :, q])
                    Eb.append(Ebq)
                    tq = tau[q]
                    if q == 0:
                        a.mul(StE[:], Eq[:], tq)
                        a.mul(StE2[:], E2q[:], tq)
                        a.mul(St2E2[:], E2q[:], tq * tq)
                        v.tensor_scalar_mul(SbtE[:], Ebq[:], tq)
                    else:
                        v.scalar_tensor_tensor(StE[:], Eq[:], tq, StE[:], ALU.mult, ALU.add)
                        v.scalar_tensor_tensor(StE2[:], E2q[:], tq, StE2[:], ALU.mult, ALU.add)
                        v.scalar_tensor_tensor(St2E2[:], E2q[:], tq * tq, St2E2[:], ALU.mult, ALU.add)
                        v.scalar_tensor_tensor(SbtE[:], Ebq[:], tq, SbtE[:], ALU.mult, ALU.add)
                        if q == 1:
                            v.tensor_add(SE[:], E[0][:], E[1][:])
                        else:
                            v.tensor_add(SE[:], SE[:], Eq[:])

                    # Pool: SE2 (own ring), SbE (Eb ring), Sb (persistent b)
                    if q == 1:
                        g.tensor_add(SE2[:], P2[0][:], P2[1][:])
                        g.tensor_add(SbE[:], Eb[0][:], Eb[1][:])
                        g.tensor_add(Sb[:], bin_[:, :, 0], bin_[:, :, 1])
                    elif q > 1:
                        g.tensor_add(SE2[:], SE2[:], P2q[:])
                        g.tensor_add(SbE[:], SbE[:], Ebq[:])
                        g.tensor_add(Sb[:], Sb[:], bin_[:, :, q])

                # Pool: gradient helpers (g0/g1/g2 alias tg planes via in-place STT later)
                g.tensor_mul(g0[:], Bv, SE[:])
                g.tensor_add(g0[:], g0[:], Sb[:])
                g.tensor_mul(g1[:], A, SE[:])
                g.tensor_mul(t3, Bv, SE2[:])
                g.tensor_sub(g1[:], g1[:], t3)
                g.tensor_sub(g1[:], g1[:], SbE[:])
                g.tensor_mul(g2[:], A, StE[:])

                a.activation(ps, SE[:], ACTF.Square)
                a.add(e[:], SE2[:], rho_ap)

                # DVE post (v8 interleaved order)
                v.tensor_mul(q2[:], Bv, StE[:])
                v.tensor_mul(f2[:], Bv, StE2[:])
                v.tensor_mul(h[:], B2[:], St2E2[:])
                a.add(h[:], h[:], rho_ap)
                a.activation(q2s[:], q2[:], ACTF.Square)
                v.tensor_mul(t1[:], SE[:], q2[:])
                v.scalar_tensor_tensor(e[:], ps, -i0, e[:], ALU.mult, ALU.add)
                v.scalar_tensor_tensor(t1[:], t1[:], i0, f2[:], ALU.mult, ALU.subtract)
                v.tensor_scalar(e[:], e[:], clamp, None, ALU.max)
                v.tensor_mul(t2a, t1[:], t1[:])
                v.scalar_tensor_tensor(h[:], q2s[:], -i0, h[:], ALU.mult, ALU.add)
                v.reciprocal(i1, e[:])
                # ghat: in-place over Pool's tg planes (out == in1)
                v.scalar_tensor_tensor(g0[:], w0, e8r, g0[:], ALU.mult, ALU.subtract)
                v.scalar_tensor_tensor(g1[:], w1, e8r, g1[:], ALU.mult, ALU.subtract)
                v.tensor_mul(t1[:], t1[:], i1)
                v.tensor_mul(t2a, t2a, i1)
                v.scalar_tensor_tensor(g0[:], A, 8.0 + e8r, g0[:], ALU.mult, ALU.add)
                v.scalar_tensor_tensor(g1[:], Bv, e8r, g1[:], ALU.mult, ALU.add)
                v.tensor_sub(h[:], h[:], t2a)
                v.tensor_sub(g2[:], g2[:], f2[:])
                v.tensor_scalar(h[:], h[:], clamp, None, ALU.max)
                v.tensor_sub(g2[:], g2[:], SbtE[:])
                a.mul(g0[:], g0[:], i0)  # z0
                v.reciprocal(i2, h[:])
                v.tensor_mul(g2[:], Bv, g2[:])
                v.scalar_tensor_tensor(g2[:], w2, e8r, g2[:], ALU.mult, ALU.add)
                v.scalar_tensor_tensor(g2[:], R1, e8r, g2[:], ALU.mult, ALU.add)

                l21 = t1
                sls = [slice(0, F // 2), slice(F // 2, F)]

                def emit(fn):
                    for sl in sls:
                        fn(sl)

                emit(lambda sl: v.tensor_mul(t2a[:, sl], SE[:, sl], g0[:, sl]))
                emit(lambda sl: v.tensor_add(g1[:, sl], g1[:, sl], t2a[:, sl]))
                emit(lambda sl: v.tensor_mul(t2b[:, sl], q2[:, sl], g0[:, sl]))
                emit(lambda sl: v.tensor_sub(g2[:, sl], g2[:, sl], t2b[:, sl]))
                emit(lambda sl: v.tensor_mul(t2a[:, sl], l21[:, sl], g1[:, sl]))
                emit(lambda sl: v.tensor_sub(g2[:, sl], g2[:, sl], t2a[:, sl]))
                emit(lambda sl: v.tensor_mul(g1[:, sl], g1[:, sl], i1[:, sl]))
                emit(lambda sl: v.tensor_mul(g2[:, sl], g2[:, sl], i2[:, sl]))
                emit(lambda sl: v.tensor_mul(t2a[:, sl], l21[:, sl], g2[:, sl]))
                emit(lambda sl: v.tensor_sub(g1[:, sl], g1[:, sl], t2a[:, sl]))
                emit(lambda sl: v.tensor_mul(t2a[:, sl], SE[:, sl], g1[:, sl]))
                emit(lambda sl: v.tensor_mul(t2b[:, sl], q2[:, sl], g2[:, sl]))
                emit(lambda sl: v.tensor_sub(t2a[:, sl], t2a[:, sl], t2b[:, sl]))
                emit(lambda sl: v.scalar_tensor_tensor(g0[:, sl], t2a[:, sl], i0, g0[:, sl], ALU.mult, ALU.add))

                v.scalar_tensor_tensor(xin[:, :, 0], g0[:], -0.125, A, ALU.mult, ALU.add)
                v.scalar_tensor_tensor(xin[:, :, 1], g1[:], -0.125, Bv, ALU.mult, ALU.add)
                v.scalar_tensor_tensor(xin[:, :, 2], g2[:], -0.125, R1, ALU.mult, ALU.add)

                if chk is not None:
                    xf = xin[:].rearrange("p f c -> p (f c)")
                    v.tensor_add(chk[:], chk[:], xf[:, 0:252])

            nc.sync.dma_start(yr[:, :, :], xin[:])
            if chk is not None:
                nc.sync.dma_start(chkd[:], chk[:])

    _split_excess_waits(nc)
    return nc

def _build_timed(tau, rho, reps):
    """Build used by test.py's timing path (same program as kernel())."""
    return _build5(tau, rho, reps=reps)


def kernel(x, z, beta, rho, sigma, b, tau):
    global LAST_RESULTS
    x = np.ascontiguousarray(np.asarray(x, dtype=np.float32).reshape(PIX, NP))
    z = np.ascontiguousarray(np.asarray(z, dtype=np.float32).reshape(PIX, NP))
    beta = np.ascontiguousarray(np.asarray(beta, dtype=np.float32).reshape(PIX, NP))
    b = np.ascontiguousarray(np.asarray(b, dtype=np.float32).reshape(PIX, NQ))
    tau_vals = np.asarray(tau, dtype=np.float32).reshape(NQ)
    rho_val = float(np.asarray(rho, dtype=np.float32).reshape(()))

    nc = _build5(tau_vals, rho_val)

    in_maps = []
    for c in range(NCORES):
        sl = slice(c * PIX_CORE, (c + 1) * PIX_CORE)
        in_maps.append(
            {
                "x": np.ascontiguousarray(x[sl]),
                "z": np.ascontiguousarray(z[sl]),
                "beta": np.ascontiguousarray(beta[sl]),
                "b": np.ascontiguousarray(b[sl]),
            }
        )

    res = run_bass_kernel_spmd(nc, in_maps, list(range(NCORES)))
    LAST_RESULTS = res
    y = np.concatenate([res.results[c]["y"] for c in range(NCORES)], axis=0)
    return y.reshape(NB, NY, NX, NP)

